# revision 77
# baseline (speedup 1.0000x reference)
"""Trainium2 Bass kernel for nn_Block_87351044866235 (sparse_attention).

Data-parallel over batch: 8 samples -> 8 NeuronCores. Channel-major
layout [C, H*W] on chip, everything fp16 on the PE (psum accumulation
stays f32). Depthwise convs run as diagonal matmuls with weights padded
to 128 columns (FWL); the qkv 1x1 is composed into its 3x3 depthwise
(27 full-matrix taps applied to yn directly); the +x residual is folded
into the pos conv's center tap. q/k are kept in plain fp16 (no hi/lo
split - fp16 logit error ~1e-3 vs ~0.07 rank gaps), transposed on the
PE, and reduced to per-head gram matrices. yn1 (padded layout), yn2 and
v stay SBUF-resident; the attn-out/proj/LN2 stage is fused into the FFN
chunk pipeline (c1) so the PE never drains between phases. LN stats use
an exactly-representable 2^-8 ones weight with 4/3 corrections folded
into existing ops. The dynamic-k gate runs at the start of phase B (one
sigmoid table load; scalar in A never leaves the sqrt table) and its
batch mean AllReduce overlaps the qkv conv + gram.
"""
import sys, os

for _p in ("/opt/trn_rl_repo", "/root/.axon_site/_ro/trn_rl_repo"):
    if os.path.isdir(_p) and _p not in sys.path:
        sys.path.append(_p)

import numpy as np
import concourse.bass as bass
import concourse.bacc as bacc
import concourse.tile as tile
from concourse import mybir
from concourse import bass_utils

try:
    from concourse import tile_utils as _tu
    _tu.max_sbuf_usage = 208 * 1024
except Exception:
    pass

dt = mybir.dt
Alu = mybir.AluOpType
Act = mybir.ActivationFunctionType
AX = mybir.AxisListType.X

EMBED, PDIM, HEADS, HID = 192, 96, 8, 256
CPH = PDIM // HEADS  # 12
SLOP = 8
RC = 3    # conv output rows per chunk
BR = 12   # rows per band (phase A x loads)

# BF16 historically named; now fp16 (same PE/DVE throughput, 11-bit mantissa)
F32, F32R, BF16 = dt.float32, dt.float32r, dt.float16


def _ceil(a, b):
    return (a + b - 1) // b


# ----------------------------------------------------------------------------
# host-side weight prep: everything 2D [partitions, free]
# ----------------------------------------------------------------------------

def _prep_weights(p):
    import ml_dtypes
    w = {}
    f32r = lambda a: (np.ascontiguousarray(a, np.float32), F32R)
    f32 = lambda a: (np.ascontiguousarray(a, np.float32), F32)
    bf16 = lambda a: (np.ascontiguousarray(a, np.float32)
                      .astype(np.float16), BF16)
    eps_bn = 1e-5

    w["ident"] = f32(np.eye(128, dtype=np.float32))

    # pos depthwise diag: [96, (t*2+cg)*128] (bf16, 128-col blocks for FWL)
    pw = p["pos_w"][:, 0]  # [192,3,3]
    pos_d = np.zeros((96, 18 * 128), np.float32)
    for t in range(9):
        dy, dx = t // 3 - 1, t % 3 - 1
        for cg in range(2):
            blk = np.diag(pw[cg * 96:(cg + 1) * 96, dy + 1, dx + 1])
            if t == 4:
                # residual +x folded into the center tap (1.0 is exact in bf16)
                blk = blk + np.eye(96, dtype=np.float32)
            pos_d[:, (t * 2 + cg) * 128:(t * 2 + cg) * 128 + 96] = blk
    w["pos_diag"] = bf16(pos_d)
    w["pos_b"] = f32(p["pos_b"].reshape(2, 96).T)  # [96, 2]

    g1v, b1v = p["ln1_g"], p["ln1_b"]
    qw = p["qkv_w"][:, :, 0, 0]  # [288, 96]
    qw_eff = qw * g1v[None, :96]
    qdw = p["qkv_dw_w"][:, 0]    # [288,3,3]
    # composed qkv conv: tap (t, j) stationary = (diag(qdw_tj) @ qw_eff_j).T
    # bias would be (sum_t qdw_tj) * (qw_j @ ln1_b); exact only with
    # zero-padded yn when that bias is 0 -- ln1_b is zeros in setup_inputs.
    qb = qw @ b1v[:96]
    assert np.abs(qb).max() < 1e-12, "composed qkv conv assumes ln1_b == 0"
    qdwc = np.zeros((96, 27 * 128), np.float32)
    for t in range(9):
        dy, dx = t // 3 - 1, t % 3 - 1
        for j in range(3):
            M = qw_eff[j * 96:(j + 1) * 96, :] \
                * qdw[j * 96:(j + 1) * 96, dy + 1, dx + 1][:, None]  # [out, in]
            qdwc[:, (t * 3 + j) * 128:(t * 3 + j) * 128 + 96] = M.T
    w["qdwc"] = bf16(qdwc)

    gw1 = p["gate_w1"][:, :, 0, 0]  # [96, 192]
    gw1_eff = gw1 * g1v[None, :]
    gw1p = np.zeros((96, 2 * 128), np.float32)
    for cg in range(2):
        gw1p[:, cg * 128:cg * 128 + 96] = gw1_eff[:, cg * 96:(cg + 1) * 96].T
    w["gate_w1T"] = bf16(gw1p)
    w["gate_b1"] = f32((p["gate_b1"] + gw1 @ b1v).reshape(96, 1))
    gw2p = np.zeros((96, 128), np.float32)
    gw2p[:, 0] = p["gate_w2"][:, :, 0, 0].reshape(96)
    w["gate_w2T"] = bf16(gw2p)
    w["gate_b2"] = f32(p["gate_b2"].reshape(1, 1))

    attn_scale = float(p["attn1"][0] + p["attn2"][0] + p["attn3"][0]
                       + p["attn4"][0])
    pj = p["proj_w"][:, :, 0, 0]
    pj1 = pj[:, :96] * attn_scale     # attn out scale folded into proj1
    pj2 = pj[:, 96:] * g1v[None, 96:]
    pj1p = np.zeros((96, 2 * 128), np.float32)
    pj2p = np.zeros((96, 2 * 128), np.float32)
    for cg in range(2):
        pj1p[:, cg * 128:cg * 128 + 96] = pj1[cg * 96:(cg + 1) * 96].T
        pj2p[:, cg * 128:cg * 128 + 96] = pj2[cg * 96:(cg + 1) * 96].T
    w["proj1T"] = bf16(pj1p)
    w["proj2T"] = bf16(pj2p)
    w["proj_bias"] = f32((pj[:, 96:] @ b1v[96:]).reshape(2, 96).T)  # [96, 2]

    w["_attn_scale"] = (attn_scale, None)
    w["tempvec"] = f32(np.repeat(p["temperature"].reshape(HEADS), CPH).reshape(96, 1))

    g2v, b2v = p["ln2_g"], p["ln2_b"]
    f1 = p["fc1_w"][:, :, 0, 0]  # [256, 192]
    f1_eff = f1 * g2v[None, :]
    fc1 = np.zeros((96, 4 * 128), np.float32)
    for mg in range(2):
        for cg in range(2):
            fc1[:, (mg * 2 + cg) * 128:(mg * 2 + cg + 1) * 128] = \
                f1_eff[mg * 128:(mg + 1) * 128, cg * 96:(cg + 1) * 96].T
    w["fc1T"] = bf16(fc1)
    w["fc1_bias"] = f32((f1 @ b2v).reshape(2, 128).T)  # [128, 2]

    s1 = p["bn1_g"] / np.sqrt(p["bn1_v"] + eps_bn)
    t1 = p["bn1_b"] - p["bn1_m"] * s1
    s2 = p["bn2_g"] / np.sqrt(p["bn2_v"] + eps_bn)
    t2 = p["bn2_b"] - p["bn2_m"] * s2
    s3 = p["bn3_g"] / np.sqrt(p["bn3_v"] + eps_bn)
    t3 = p["bn3_b"] - p["bn3_m"] * s3

    dw1w, dw2w, dw3w = p["dw1_w"][:, 0], p["dw2_w"][:, 0], p["dw3_w"][:, 0]
    dw1b, dw2b, dw3b = p["dw1_b"], p["dw2_b"], p["dw3_b"]
    s1g = [s1[i * 64:(i + 1) * 64] for i in range(4)]
    t1g = [t1[i * 64:(i + 1) * 64] for i in range(4)]

    pair_d = np.zeros((128, 25 * 128), np.float32)
    for t in range(25):
        dy, dx = t // 5 - 2, t % 5 - 2
        blk = np.zeros((128, 128), np.float32)
        d2 = dw2w[:, dy + 2, dx + 2] * s1g[2]
        if dy == 0 and dx == 0:
            d2 = d2 + s1g[2]
        blk[64:, 64:] = np.diag(d2)
        if -1 <= dy <= 1 and -1 <= dx <= 1:
            d1 = dw1w[:, dy + 1, dx + 1] * s1g[1]
            if dy == 0 and dx == 0:
                d1 = d1 + s1g[1]
            blk[:64, :64] = np.diag(d1)
        pair_d[:, t * 128:(t + 1) * 128] = blk
    w["pair_diag"] = bf16(pair_d)
    bc1 = t1g[1] * dw1w.sum((1, 2)) + dw1b + t1g[1]
    bc2 = t1g[2] * dw2w.sum((1, 2)) + dw2b + t1g[2]
    w["pair_bias"] = f32(np.concatenate([bc1, bc2]).reshape(128, 1))

    # rows 64:128 of v0t2 hold the same data stored shifted +1, so a read at
    # AP offset (dy, dxa) yields tap (dy, dxa-1) for those rows.
    dw3_passes = []
    for dy in range(-3, 4):
        for dxa in (-2, 0, 2):
            dw3_passes.append((dy, dxa, True))
        dw3_passes.append((dy, 3, False))
    dw3_d = np.zeros((128, len(dw3_passes) * 128), np.float32)
    for i, (dy, dxa, hasb) in enumerate(dw3_passes):
        wa = dw3w[:, dy + 3, dxa + 3] * s1g[3]
        if dy == 0 and dxa == 0:
            wa = wa + s1g[3]
        dw3_d[:64, i * 128:i * 128 + 64] = np.diag(wa)
        if hasb:
            wb = dw3w[:, dy + 3, dxa - 1 + 3] * s1g[3]
            if dy == 0 and dxa - 1 == 0:
                wb = wb + s1g[3]
            dw3_d[64:, i * 128:i * 128 + 64] = np.diag(wb)
    w["dw3_diag"] = bf16(dw3_d)
    w["_dw3_passes"] = (dw3_passes, None)
    w["dw3_bias"] = f32((t1g[3] * dw3w.sum((1, 2)) + dw3b + t1g[3]).reshape(64, 1))

    d0w, d0b = p["dw0_w"][:, 0, 0, 0], p["dw0_b"]
    w["g0_scale"] = f32(((d0w + 1.0) * s1g[0]).reshape(64, 1))
    w["g0_bias"] = f32(((d0w + 1.0) * t1g[0] + d0b).reshape(64, 1))

    f2 = p["fc2_w"][:, :, 0, 0]  # [192, 256]
    f2a = f2 * s2[None, :]
    f2b = f2 * (t2 * s1)[None, :]
    cstv = f2 @ (t2 * t1)
    fc2a = np.zeros((128, 4 * 128), np.float32)
    for cg in range(2):
        for kg in range(2):
            fc2a[:, (cg * 2 + kg) * 128:(cg * 2 + kg) * 128 + 96] = \
                f2a[cg * 96:(cg + 1) * 96, kg * 128:(kg + 1) * 128].T
    w["fc2aT"] = bf16(fc2a)
    fb0 = np.zeros((64, 2 * 128), np.float32)
    fb12 = np.zeros((128, 2 * 128), np.float32)
    fb3 = np.zeros((64, 2 * 128), np.float32)
    for cg in range(2):
        fb0[:, cg * 128:cg * 128 + 96] = f2b[cg * 96:(cg + 1) * 96, 0:64].T
        fb12[:, cg * 128:cg * 128 + 96] = f2b[cg * 96:(cg + 1) * 96, 64:192].T
        fb3[:, cg * 128:cg * 128 + 96] = f2b[cg * 96:(cg + 1) * 96, 192:256].T
    w["fc2bT_g0"] = bf16(fb0)
    w["fc2bT_g12"] = bf16(fb12)
    w["fc2bT_g3"] = bf16(fb3)
    w["s3v"] = f32(np.stack([s3[:96], s3[96:]], axis=1))          # [96, 2]
    # out = s3*(psum + cst) + t3 + xc' ; psum excludes cst, so bias = s3*cst + t3
    ob = s3 * cstv + t3
    w["out_bias"] = f32(np.stack([ob[:96], ob[96:]], axis=1))     # [96, 2]

    sg = np.where(s1 == 0, 1.0, s1)
    padv = -t1 / sg
    w["padv1"] = f32(np.concatenate([padv[64:128], padv[128:192]]).reshape(128, 1))
    w["padv2"] = f32(np.concatenate([padv[192:256], padv[192:256]]).reshape(128, 1))
    w["s1a"] = f32(s1[:128].reshape(128, 1))
    w["s1b"] = f32(s1[128:].reshape(128, 1))
    w["t1a"] = f32(t1[:128].reshape(128, 1))
    w["t1b"] = f32(t1[128:].reshape(128, 1))

    # bf16 stats weights: exactly-representable 2^-8; the (EMBED/256)
    # correction factor 4/3 is applied in the finishing ops on-chip
    ones = np.full((96, 128), 2.0 ** -8, np.float32)
    w["ones_st"] = bf16(ones)
    w["ones_st2"] = bf16(ones)
    w["epsv"] = f32(np.full((128, 1), 1e-6, np.float32))
    w["epsv2"] = f32(np.full((128, 1), 1e-6, np.float32))
    vm = np.zeros((96, 96), np.float32)
    for h in range(HEADS):
        vm[h * CPH:(h + 1) * CPH, h * CPH:(h + 1) * CPH] = 1.0
    w["vmask"] = f32(vm)
    w["vmaskb"] = bf16((vm - 1.0) * 60.0)
    return w


WSPEC = {
    "ident": ([128, 128], F32), "pos_diag": ([96, 18 * 128], BF16),
    "pos_b": ([96, 2], F32), "qdwc": ([96, 27 * 128], BF16),
    "gate_w1T": ([96, 2 * 128], BF16), "gate_b1": ([96, 1], F32),
    "gate_w2T": ([96, 128], BF16), "gate_b2": ([1, 1], F32),
    "proj1T": ([96, 2 * 128], BF16), "proj2T": ([96, 2 * 128], BF16),
    "proj_bias": ([96, 2], F32), "tempvec": ([96, 1], F32),
    "fc1T": ([96, 4 * 128], BF16), "fc1_bias": ([128, 2], F32),
    "pair_diag": ([128, 25 * 128], BF16), "pair_bias": ([128, 1], F32),
    "dw3_diag": ([128, 28 * 128], BF16), "dw3_bias": ([64, 1], F32),
    "g0_scale": ([64, 1], F32), "g0_bias": ([64, 1], F32),
    "fc2aT": ([128, 4 * 128], BF16), "fc2bT_g0": ([64, 2 * 128], BF16),
    "fc2bT_g12": ([128, 2 * 128], BF16), "fc2bT_g3": ([64, 2 * 128], BF16),
    "s3v": ([96, 2], F32), "out_bias": ([96, 2], F32),
    "padv1": ([128, 1], F32),
    "padv2": ([128, 1], F32),
    "s1a": ([128, 1], F32), "s1b": ([128, 1], F32),
    "t1a": ([128, 1], F32), "t1b": ([128, 1], F32),
    "ones_st": ([96, 128], BF16), "ones_st2": ([96, 128], BF16),
    "epsv": ([128, 1], F32), "epsv2": ([128, 1], F32),
    "vmask": ([96, 96], F32), "vmaskb": ([96, 96], BF16),
}

# weights only used by the fused attn-out/FFN phase (loaded during phase A)
C_ONLY_W = ['proj1T', 'proj2T', 'proj_bias', 'fc1T', 'fc1_bias', 'pair_diag',
            'pair_bias', 'dw3_diag', 'dw3_bias', 'g0_scale', 'g0_bias',
            'fc2aT', 'fc2bT_g0', 'fc2bT_g12', 'fc2bT_g3', 's3v', 'out_bias',
            's1a', 's1b', 't1a', 't1b', 'padv1', 'padv2', 'ones_st2', 'epsv2']


# ----------------------------------------------------------------------------
# device kernel
# ----------------------------------------------------------------------------

def build(nc, H, W, n_cores, attn_scale, dw3_passes):
    S = H * W
    Wp1 = W + 2
    P1B = (BR + 2) * Wp1 + 2 * SLOP   # phase-A x band buffer (pad1)
    YB = 8                            # guard cells before yn1p row 0
    P1Y = YB + (H + 3) * Wp1          # full padded yn1 image (+top/bot pad rows)
    Wp3, Hp3 = W + 6, H + 6
    P3 = Hp3 * Wp3 + 2 * SLOP
    NCH = _ceil(H, RC)
    NB = _ceil(H, BR)

    x_t = nc.dram_tensor("x", [EMBED, S], BF16, kind="ExternalInput")
    out_t = nc.dram_tensor("out", [EMBED, S], F32, kind="ExternalOutput")
    wt = {k: nc.dram_tensor("w_" + k, shp, d, kind="ExternalInput")
          for k, (shp, d) in WSPEC.items()}

    def pd3(r):
        return SLOP + r * Wp3

    with tile.TileContext(nc) as tc:
        with (
            tc.tile_pool(name="dram", bufs=1, space="DRAM") as dram,
            tc.tile_pool(name="persist", bufs=1) as pers,
        ):
            ws = {}

            def _load_w(pool, names):
                for k in names:
                    shp, d = WSPEC[k]
                    tl = pool.tile(shp, d, tag="w_" + k, name="w_" + k)
                    nc.sync.dma_start(out=tl[:], in_=wt[k][:])
                    ws[k] = tl

            xc_sp = [dram.tile([96, S], BF16, name=f"xc_sp{i}") for i in range(2)]
            cc_in = dram.tile([1, 1], F32)
            cc_out = dram.tile([1, 1], F32)

            gsum = pers.tile([1, 64], F32)
            nc.vector.memset(gsum[:], 0.0)
            # per-chunk q/k squared-norm partials (cols 0:64 q, 64:128 k)
            nrm2a = pers.tile([96, 128], F32)
            dynk = pers.tile([96, 1], F32)
            probsT = pers.tile([96, 128], BF16)
            identb = pers.tile([96, 96], BF16)
            yn2res = pers.tile([96, S], BF16)   # LN1 out, ch 96:192 (flat)
            v_res = pers.tile([96, S], BF16)    # attn v image (flat)

            # phase-C weight pool opens first (released last, LIFO); its
            # DMAs are emitted at the end of phase A
            _wpc_cm = tc.tile_pool(name="wpC", bufs=1)
            wpc = _wpc_cm.__enter__()
            _wpab_cm = tc.tile_pool(name="wpAB", bufs=1)
            wpab = _wpab_cm.__enter__()
            _ab_names = [k for k in WSPEC if k not in C_ONLY_W]
            # pos weights first so the first conv matmul isn't DMA-gated
            _ab_names.sort(key=lambda k: 0 if k in ("pos_diag", "pos_b", "ones_st",
                                                    "epsv") else 1)
            _load_w(wpab, _ab_names)
            # padded resident yn1 image (LN1 out ch 0:96), bf16; its memset
            # is deferred into band 0 so it doesn't head the vector queue
            # and delay the first x DMA
            _ynp_cm = tc.tile_pool(name="ynp", bufs=1)
            ynp = _ynp_cm.__enter__()
            yn1p = ynp.tile([96, P1Y], BF16)

            # ================= PHASE A =================
            ci_g = 0
            with (
                tc.tile_pool(name="pa_band", bufs=2) as pab,
                tc.tile_pool(name="pa_rot", bufs=3) as par,
                tc.tile_pool(name="pa_ps", bufs=2, space="PSUM") as paps,
                tc.tile_pool(name="pa_psg", bufs=2, space="PSUM") as papsg,
            ):
                for b in range(NB):
                    r0, r1 = b * BR, min((b + 1) * BR, H)
                    ylo, yhi = max(r0 - 1, 0), min(r1 + 1, H)
                    xband = [pab.tile([96, P1B], BF16, tag=f"xb{cg}", name=f"xb{cg}")
                             for cg in range(2)]
                    for cg in range(2):
                        if b < 2:
                            nc.vector.memset(xband[cg][:].bitcast(F32), 0.0)
                        elif b == NB - 1 and yhi - (r0 - 1) < BR + 2:
                            o = SLOP + (yhi - (r0 - 1)) * Wp1
                            nc.vector.memset(xband[cg][:, o:].bitcast(F32), 0.0)
                        dst = xband[cg][:, SLOP + (ylo - (r0 - 1)) * Wp1:
                                        SLOP + (yhi - (r0 - 1)) * Wp1] \
                            .rearrange("p (r w) -> p r w", w=Wp1)[:, :, 1:1 + W]
                        nc.sync.dma_start(
                            out=dst,
                            in_=x_t[cg * 96:(cg + 1) * 96, ylo * W:yhi * W]
                            .rearrange("c (r w) -> c r w", w=W))
                    if b == 0:
                        # deferred init work overlaps the band-0 x DMA
                        nc.vector.memset(yn1p[:].bitcast(F32), 0.0)
                        nc.vector.memset(probsT[:].bitcast(F32), 0.0)
                        nc.vector.memset(nrm2a[:], 0.0)
                        nc.vector.tensor_copy(out=identb[:],
                                              in_=ws["ident"][:96, :96])
                    for c0 in range(r0, r1, RC):
                        nr_c = min(RC, H - c0)
                        N = nr_c * Wp1
                        NN = nr_c * W
                        sb0 = SLOP + (c0 - r0 + 1) * Wp1
                        xc_ch = [par.tile([96, RC * W], BF16, tag=f"xc{cg}",
                                          name=f"xc{cg}") for cg in range(2)]
                        xsq = [par.tile([96, RC * W], BF16, tag=f"xq{cg}",
                                        name=f"xq{cg}") for cg in range(2)]
                        for cg in range(2):
                            ps = paps.tile([128, RC * Wp1], F32, tag="posps", bufs=3)
                            for t in range(9):
                                dy, dx = t // 3 - 1, t % 3 - 1
                                o = sb0 + dy * Wp1 + dx
                                nc.tensor.matmul(
                                    ps[:, :N],
                                    ws["pos_diag"][:, (t * 2 + cg) * 128:(t * 2 + cg + 1) * 128],
                                    xband[cg][:, o:o + N],
                                    start=(t == 0), stop=(t == 8))
                            ps_int = ps[:96, :N].rearrange("p (r w) -> p r w", w=Wp1)[:, :, 1:1 + W]
                            xcv = xc_ch[cg][:, :NN].rearrange("p (r w) -> p r w", w=W)
                            nc.scalar.activation(xcv, ps_int, Act.Identity,
                                                 bias=ws["pos_b"][:, cg:cg + 1])
                            nc.scalar.square(xsq[cg][:, :NN], xc_ch[cg][:, :NN])
                            nc.sync.dma_start(out=xc_sp[cg][:, c0 * W:c0 * W + NN],
                                              in_=xc_ch[cg][:, :NN])
                        mu_ps = paps.tile([128, RC * W], F32, tag="mups")
                        m2_ps = paps.tile([128, RC * W], F32, tag="m2ps")
                        for cg in range(2):
                            nc.tensor.matmul(mu_ps[:, :NN], ws["ones_st"], xc_ch[cg][:, :NN],
                                             start=(cg == 0), stop=(cg == 1))
                            nc.tensor.matmul(m2_ps[:, :NN], ws["ones_st"], xsq[cg][:, :NN],
                                             start=(cg == 0), stop=(cg == 1))
                        musq = par.tile([128, RC * W], F32, tag="musq")
                        nc.scalar.activation(musq[:, :NN], mu_ps[:, :NN], Act.Square,
                                             scale=4.0 / 3.0)
                        var = par.tile([128, RC * W], F32, tag="var")
                        nc.vector.scalar_tensor_tensor(
                            out=var[:, :NN], in0=m2_ps[:, :NN], scalar=4.0 / 3.0,
                            in1=musq[:, :NN], op0=Alu.mult, op1=Alu.subtract)
                        sd = par.tile([128, RC * W], F32, tag="sd")
                        nc.scalar.activation(sd[:, :NN], var[:, :NN], Act.Sqrt, bias=ws["epsv"])
                        rstd = par.tile([128, RC * W], F32, tag="rstd")
                        nc.vector.reciprocal_approx_fast(out=rstd[:, :NN], in_=sd[:, :NN])
                        # cg0: normalize straight into the padded resident yn1
                        td0 = par.tile([96, RC * W], F32, tag="td0")
                        nc.vector.scalar_tensor_tensor(
                            out=td0[:, :NN], in0=mu_ps[:96, :NN], scalar=-4.0 / 3.0,
                            in1=xc_ch[0][:, :NN], op0=Alu.mult, op1=Alu.add)
                        yb0 = YB + (c0 + 1) * Wp1 + 1
                        nc.vector.tensor_tensor(
                            out=yn1p[:, yb0:yb0 + nr_c * Wp1]
                            .rearrange("p (r w) -> p r w", w=Wp1)[:, :, 0:W],
                            in0=td0[:, :NN].rearrange("p (r w) -> p r w", w=W),
                            in1=rstd[:96, :NN].rearrange("p (r w) -> p r w", w=W),
                            op=Alu.mult)
                        # cg1: straight into the flat resident yn2 image
                        td1 = par.tile([96, RC * W], F32, tag="td1")
                        nc.vector.scalar_tensor_tensor(
                            out=td1[:, :NN], in0=mu_ps[:96, :NN], scalar=-4.0 / 3.0,
                            in1=xc_ch[1][:, :NN], op0=Alu.mult, op1=Alu.add)
                        nc.vector.tensor_tensor(out=yn2res[:, c0 * W:c0 * W + NN],
                                                in0=td1[:, :NN],
                                                in1=rstd[:96, :NN], op=Alu.mult)

                # phase-C weight DMAs overlap phase B compute
                _load_w(wpc, C_ONLY_W)

            # ================= PHASE B =================
            # composed qkv 3x3 conv (27 full-matrix taps) straight off yn1p;
            # hi/lo bf16 split; DMA-xbar transposes into qkband; gram matmuls.
            with (
                tc.tile_pool(name="pb_rot", bufs=3) as pbr,
                tc.tile_pool(name="gram_ps", bufs=1, space="PSUM") as gpsp,
            ):
                g1_ps = gpsp.tile([96, 96], F32)
                with (
                    tc.tile_pool(name="pb_ps", bufs=2, space="PSUM") as pbps,
                    tc.tile_pool(name="pb_pst", bufs=2, space="PSUM") as pbpst,
                ):
                    # dynamic-k gate (here instead of phase A so the scalar
                    # engine never swaps between the sqrt and sigmoid tables)
                    for ci in range(NCH):
                        c0 = ci * RC
                        nr_c = min(RC, H - c0)
                        NN = nr_c * W
                        gps = pbps.tile([128, RC * W], F32, tag="gps", bufs=2)
                        for rr in range(nr_c):
                            yb0 = YB + (c0 + rr + 1) * Wp1 + 1
                            nc.tensor.matmul(gps[:, rr * W:(rr + 1) * W],
                                             ws["gate_w1T"][:, 0:128],
                                             yn1p[:, yb0:yb0 + W],
                                             start=True, stop=False,
                                             skip_group_check=True)
                        nc.tensor.matmul(gps[:, :NN], ws["gate_w1T"][:, 128:256],
                                         yn2res[:, c0 * W:c0 * W + NN],
                                         start=False, stop=True,
                                         skip_group_check=True)
                        g1s = pbr.tile([96, RC * W], BF16, tag="g1s")
                        nc.scalar.activation(g1s[:, :NN], gps[:96, :NN], Act.Relu,
                                             bias=ws["gate_b1"])
                        g2ps = pbps.tile([128, RC * W], F32, tag="gps", bufs=2)
                        nc.tensor.matmul(g2ps[:, :NN], ws["gate_w2T"], g1s[:, :NN],
                                         start=True, stop=True)
                        sgt = pbr.tile([1, RC * W], F32, tag="sgt")
                        nc.scalar.activation(sgt[:, :NN], g2ps[:1, :NN], Act.Sigmoid,
                                             bias=ws["gate_b2"],
                                             accum_out=gsum[0:1, ci:ci + 1])
                    # gate mean -> AllReduce -> dynk (overlaps the conv + gram)
                    gred = pers.tile([1, 1], F32)
                    nc.vector.reduce_sum(gred[:], gsum[0:1, 0:NCH], axis=AX)
                    gsc = pers.tile([1, 1], F32)
                    nc.vector.tensor_scalar_mul(gsc[:], gred[:],
                                                float(CPH) / (n_cores * S))
                    nc.sync.dma_start(out=cc_in[:], in_=gsc[:])
                    nc.gpsimd.collective_compute(
                        "AllReduce", Alu.add,
                        replica_groups=[list(range(n_cores))],
                        ins=[cc_in.opt()], outs=[cc_out.opt()])
                    nc.sync.dma_start(out=dynk[:],
                                      in_=cc_out[:].partition_broadcast(96))
                    for ci in range(NCH):
                        c0 = ci * RC
                        nr_c = min(RC, H - c0)
                        N = nr_c * Wp1
                        NN = nr_c * W
                        sb0 = YB + (c0 + 1) * Wp1
                        qkf = {}
                        for j in range(3):
                            ps = pbps.tile([128, RC * Wp1], F32, tag="dwps", bufs=3)
                            for t in range(9):
                                dy, dx = t // 3 - 1, t % 3 - 1
                                o = sb0 + dy * Wp1 + dx
                                nc.tensor.matmul(
                                    ps[:, :N],
                                    ws["qdwc"][:, (t * 3 + j) * 128:(t * 3 + j + 1) * 128],
                                    yn1p[:, o:o + N],
                                    start=(t == 0), stop=(t == 8))
                            ps_int = ps[:96, :N].rearrange("p (r w) -> p r w", w=Wp1)[:, :, 1:1 + W]
                            if j == 2:
                                nc.scalar.copy(
                                    v_res[:, c0 * W:c0 * W + NN]
                                    .rearrange("p (r w) -> p r w", w=W), ps_int)
                            else:
                                qf = pbr.tile([96, RC * W], BF16, tag=f"qf{j}")
                                nc.scalar.copy(
                                    qf[:, :NN].rearrange("p (r w) -> p r w", w=W),
                                    ps_int)
                                qkf[j] = qf
                                # ||q||^2 / ||k||^2 partials on the idle B
                                # scalar engine (replaces the qq/kk grams)
                                sqs = pbr.tile([96, RC * W], BF16, tag="sqs")
                                nc.scalar.activation(
                                    sqs[:, :NN], qf[:, :NN], Act.Square,
                                    accum_out=nrm2a[:, j * 64 + ci:j * 64 + ci + 1])
                        qkb = pbr.tile([128, RC * 192], BF16, tag="qkb", bufs=3)
                        for rr in range(nr_c):
                            rl = rr * W
                            ro = rr * 192
                            tps = pbpst.tile([W, 192], BF16, tag="qktp")
                            for idx in range(2):
                                nc.tensor.transpose(
                                    tps[:, idx * 96:(idx + 1) * 96],
                                    qkf[idx][:, rl:rl + W], identb[:])
                            nc.vector.tensor_copy(out=qkb[:, ro:ro + 192],
                                                  in_=tps[:])
                        for rr in range(nr_c):
                            ro = rr * 192
                            first, last = (c0 + rr == 0), (c0 + rr == H - 1)
                            nc.tensor.matmul(g1_ps[:], qkb[:, ro:ro + 96],
                                             qkb[:, ro + 96:ro + 192],
                                             start=first, stop=last)

                # ---- attn block ----
                ident = ws["ident"]
                with (
                    tc.tile_pool(name="at_ps", bufs=2, space="PSUM") as atps,
                    tc.tile_pool(name="at_sb", bufs=1) as ab,
                ):
                    g1sb = ab.tile([96, 96], F32)
                    nc.scalar.copy(g1sb[:], g1_ps[:])
                    nq2 = ab.tile([96, 1], F32)
                    nc.vector.reduce_sum(nq2[:], nrm2a[:, 0:NCH], axis=AX)
                    nk2 = ab.tile([96, 1], F32)
                    nc.vector.reduce_sum(nk2[:], nrm2a[:, 64:64 + NCH], axis=AX)

                    def rsqrt_clamped(nm, src):
                        sq = ab.tile([96, 1], F32, tag=nm + "sq")
                        nc.scalar.sqrt(sq[:], src[:])
                        cl = ab.tile([96, 1], F32, tag=nm + "cl")
                        nc.vector.tensor_scalar_max(cl[:], sq[:], 1e-12)
                        rvv = ab.tile([96, 1], F32, tag=nm)
                        nc.vector.reciprocal(rvv[:], cl[:])
                        return rvv

                    rq = rsqrt_clamped("rq", nq2)
                    rk = rsqrt_clamped("rk", nk2)
                    rqt = ab.tile([96, 1], F32)
                    nc.vector.tensor_tensor(out=rqt[:], in0=rq[:], in1=ws["tempvec"][:],
                                            op=Alu.mult)
                    asr = ab.tile([96, 96], F32)
                    nc.vector.tensor_scalar_mul(asr[:], g1sb[:], rqt[:])
                    as_ps = atps.tile([96, 96], F32, tag="atp")
                    nc.tensor.transpose(as_ps[:], asr[:], ident[:96, :96])
                    ast = ab.tile([96, 96], F32)
                    nc.vector.tensor_scalar_mul(ast[:], as_ps[:], rk[:])
                    as2_ps = atps.tile([96, 96], F32, tag="atp")
                    nc.tensor.transpose(as2_ps[:], ast[:], ident[:96, :96])
                    as2 = ab.tile([96, 96], F32)
                    nc.scalar.copy(as2[:], as2_ps[:])
                    # mask off-head-block entries to -60 on UNSHIFTED logits
                    # (fp16 ranks need full resolution near zero)
                    t0m = ab.tile([96, 96], BF16)
                    nc.vector.tensor_tensor(out=t0m[:], in0=as2[:], in1=ws["vmask"][:],
                                            op=Alu.mult)
                    amf = ab.tile([96, 96], BF16)
                    nc.vector.tensor_tensor(out=amf[:], in0=t0m[:], in1=ws["vmaskb"][:],
                                            op=Alu.add)
                    # rank+1 over full row via pairwise is_ge (16-bit: 2x DVE)
                    rnk3 = ab.tile([96, 96 * 96], BF16)
                    a_i = amf[:].unsqueeze(1).broadcast_to([96, 96, 96])
                    a_d = amf[:].unsqueeze(2).broadcast_to([96, 96, 96])
                    rvw = rnk3[:].rearrange("p (i d) -> p i d", d=96)
                    nc.vector.tensor_tensor(out=rvw, in0=a_i, in1=a_d, op=Alu.is_ge)
                    rank1 = ab.tile([96, 96], F32)
                    nc.vector.reduce_sum(rank1[:].unsqueeze(2), rvw, axis=AX)
                    sel = ab.tile([96, 96], F32)
                    nc.vector.tensor_tensor(out=sel[:], in0=rank1[:],
                                            in1=dynk[:].broadcast_to([96, 96]), op=Alu.is_le)
                    t60 = ab.tile([96, 96], F32)
                    nc.vector.tensor_scalar_add(t60[:], amf[:], 60.0)
                    am = ab.tile([96, 96], F32)
                    nc.vector.tensor_tensor(out=am[:], in0=t60[:], in1=sel[:], op=Alu.mult)
                    nc.vector.tensor_scalar_add(am[:], am[:], -60.0)
                    mx = ab.tile([96, 1], F32)
                    nc.vector.reduce_max(mx[:], am[:], axis=AX)
                    nmx = ab.tile([96, 1], F32)
                    nc.vector.tensor_scalar_mul(nmx[:], mx[:], -1.0)
                    ex = ab.tile([96, 96], F32)
                    nc.scalar.activation(ex[:], am[:], Act.Exp, bias=nmx[:])
                    sme = ab.tile([96, 1], F32)
                    nc.vector.reduce_sum(sme[:], ex[:], axis=AX)
                    rsm = ab.tile([96, 1], F32)
                    nc.vector.reciprocal(rsm[:], sme[:])
                    probs = ab.tile([96, 96], F32)
                    nc.vector.tensor_scalar_mul(probs[:], ex[:], rsm[:])
                    pt_ps = atps.tile([96, 96], F32, tag="atp2")
                    nc.tensor.transpose(pt_ps[:], probs[:], ident[:96, :96])
                    nc.scalar.copy(probsT[:, 0:96], pt_ps[:])

            _ynp_cm.__exit__(None, None, None)
            _wpab_cm.__exit__(None, None, None)

            # ========== PHASE C (attn-out/proj/LN2 fused with FFN pipeline) ==
            with tc.tile_pool(name="c_v0", bufs=1) as cv0:
                v0t1 = cv0.tile([128, P3], BF16)
                v0t2 = cv0.tile([128, P3], BF16)
                with (
                    tc.tile_pool(name="c1_rot", bufs=2) as c1r,
                    tc.tile_pool(name="c2_rot", bufs=2) as c2r,
                    tc.tile_pool(name="c_ps", bufs=1, space="PSUM") as cps,
                ):
                    # pad cells must hold -t1/s1 so the bn-folded depthwise
                    # reads zeros in v0_bn space at image borders; only the
                    # border cells need init (interior is overwritten below)
                    head_end = pd3(3) + 4
                    tail_beg = pd3(H + 3) - 3
                    mid0 = pd3(3) + W + 3
                    for v0t, padv in ((v0t1, "padv1"), (v0t2, "padv2")):
                        # memset must run on f32 views; for bf16 tiles use
                        # even-aligned (wider) regions -- the extra cells are
                        # data cells the fc1 writes below overwrite anyway
                        f32v = v0t[:].bitcast(F32)
                        msets = (
                            f32v[:, 0:(head_end + 1) // 2],
                            f32v[:, (tail_beg - 1) // 2:P3 // 2],
                            f32v[:, (mid0 - 1) // 2:(mid0 - 1) // 2 + (H - 1) * (Wp3 // 2)]
                            .rearrange("p (r w) -> p r w", w=Wp3 // 2)[:, :, 0:4],
                        )
                        for reg in msets:
                            nc.vector.memset(reg, 0.0)
                        for reg in (
                            v0t[:, 0:head_end],
                            v0t[:, tail_beg:P3],
                            v0t[:, mid0:mid0 + (H - 1) * Wp3].rearrange(
                                "p (r w) -> p r w", w=Wp3)[:, :, 0:7],
                        ):
                            nc.vector.tensor_scalar_add(reg, reg, ws[padv])
                    xp_hist = {}
                    vg0_hist = {}

                    def emit_c1(ci):
                        c0 = ci * RC
                        nr_c = min(RC, H - c0)
                        NN = nr_c * W
                        o0 = c0 * W
                        # residual stream chunks
                        xpch = [c1r.tile([96, RC * W], BF16, tag=f"cx{cg}",
                                         name=f"cx{cg}", bufs=3) for cg in range(2)]
                        xp_hist[ci] = xpch
                        xcch = [c1r.tile([96, RC * W], BF16, tag=f"cxc{cg}",
                                         name=f"cxc{cg}") for cg in range(2)]
                        for cg in range(2):
                            nc.sync.dma_start(out=xcch[cg][:, :NN],
                                              in_=xc_sp[cg][:, o0:o0 + NN])
                        # attention out chunk
                        av_ps = cps.tile([128, RC * W], F32, tag="avpj", bufs=2)
                        nc.tensor.matmul(av_ps[:, :NN], probsT[:],
                                         v_res[:, o0:o0 + NN], start=True, stop=True)
                        avs = c1r.tile([96, RC * W], BF16, tag="avs")
                        nc.vector.tensor_copy(out=avs[:, :NN], in_=av_ps[:96, :NN])
                        xsq = [c1r.tile([96, RC * W], BF16, tag=f"cs{cg}",
                                        name=f"cs{cg}") for cg in range(2)]
                        for cg in range(2):
                            pj_ps = cps.tile([128, RC * W], F32, tag="avpj", bufs=2)
                            nc.tensor.matmul(pj_ps[:, :NN],
                                             ws["proj2T"][:, cg * 128:(cg + 1) * 128],
                                             yn2res[:, o0:o0 + NN], start=True, stop=False)
                            nc.tensor.matmul(pj_ps[:, :NN],
                                             ws["proj1T"][:, cg * 128:(cg + 1) * 128],
                                             avs[:, :NN], start=False, stop=True)
                            nc.vector.scalar_tensor_tensor(
                                out=xpch[cg][:, :NN], in0=pj_ps[:96, :NN],
                                scalar=ws["proj_bias"][:, cg:cg + 1], in1=xcch[cg][:, :NN],
                                op0=Alu.add, op1=Alu.add)
                            nc.scalar.square(xsq[cg][:, :NN], xpch[cg][:, :NN])
                        # LN2 stats
                        mu_ps = cps.tile([128, RC * W], F32, tag="c1s", bufs=2)
                        m2_ps = cps.tile([128, RC * W], F32, tag="c1s", bufs=2)
                        for cg in range(2):
                            nc.tensor.matmul(mu_ps[:, :NN], ws["ones_st2"], xpch[cg][:, :NN],
                                             start=(cg == 0), stop=(cg == 1))
                            nc.tensor.matmul(m2_ps[:, :NN], ws["ones_st2"], xsq[cg][:, :NN],
                                             start=(cg == 0), stop=(cg == 1))
                        musq = c1r.tile([128, RC * W], F32, tag="cst")
                        nc.scalar.activation(musq[:, :NN], mu_ps[:, :NN], Act.Square,
                                             scale=4.0 / 3.0)
                        var = c1r.tile([128, RC * W], F32, tag="cst")
                        nc.vector.scalar_tensor_tensor(
                            out=var[:, :NN], in0=m2_ps[:, :NN], scalar=4.0 / 3.0,
                            in1=musq[:, :NN], op0=Alu.mult, op1=Alu.subtract)
                        sd = c1r.tile([128, RC * W], F32, tag="cst")
                        nc.scalar.activation(sd[:, :NN], var[:, :NN], Act.Sqrt,
                                             bias=ws["epsv2"])
                        rstd = c1r.tile([128, RC * W], F32, tag="cst")
                        nc.vector.reciprocal_approx_fast(out=rstd[:, :NN], in_=sd[:, :NN])
                        yn2t = [c1r.tile([96, RC * W], BF16, tag=f"cy{cg}",
                                         name=f"cy{cg}") for cg in range(2)]
                        for cg in range(2):
                            td = c1r.tile([96, RC * W], F32, tag="ctd")
                            nc.vector.scalar_tensor_tensor(
                                out=td[:, :NN], in0=mu_ps[:96, :NN], scalar=-4.0 / 3.0,
                                in1=xpch[cg][:, :NN], op0=Alu.mult, op1=Alu.add)
                            nc.vector.tensor_tensor(out=yn2t[cg][:, :NN], in0=td[:, :NN],
                                                    in1=rstd[:96, :NN], op=Alu.mult)
                        base = pd3(3 + c0) + 3

                        def v0dst(v0t, lo, hi, off=0):
                            return v0t[lo:hi, base + off:base + off + nr_c * Wp3] \
                                .rearrange("p (r w) -> p r w", w=Wp3)[:, :, 0:W]

                        vg0 = c1r.tile([64, RC * W], BF16, tag="vg0", bufs=3)
                        vg0_hist[ci] = vg0
                        for mg in range(2):
                            fps = cps.tile([128, RC * W], F32, tag="c1s", bufs=2)
                            for cg in range(2):
                                nc.tensor.matmul(
                                    fps[:, :NN],
                                    ws["fc1T"][:, (mg * 2 + cg) * 128:(mg * 2 + cg + 1) * 128],
                                    yn2t[cg][:, :NN], start=(cg == 0), stop=(cg == 1))
                            fsrc = lambda lo, hi: fps[lo:hi, :NN].rearrange(
                                "p (r w) -> p r w", w=W)
                            if mg == 0:
                                nc.scalar.activation(vg0[:, :NN], fps[0:64, :NN], Act.Gelu,
                                                     bias=ws["fc1_bias"][0:64, 0:1])
                                nc.scalar.activation(v0dst(v0t1, 0, 64), fsrc(64, 128),
                                                     Act.Gelu, bias=ws["fc1_bias"][64:128, 0:1])
                            else:
                                nc.scalar.activation(v0dst(v0t1, 64, 128), fsrc(0, 64),
                                                     Act.Gelu, bias=ws["fc1_bias"][0:64, 1:2])
                                nc.scalar.activation(v0dst(v0t2, 0, 64), fsrc(64, 128),
                                                     Act.Gelu, bias=ws["fc1_bias"][64:128, 1:2])
                                nc.scalar.activation(v0dst(v0t2, 64, 128, off=1), fsrc(64, 128),
                                                     Act.Gelu, bias=ws["fc1_bias"][64:128, 1:2])

                    def emit_c2(ci):
                        c0 = ci * RC
                        nr_c = min(RC, H - c0)
                        N = nr_c * Wp3
                        NN = nr_c * W
                        sb0 = pd3(3 + c0)
                        ps_a = cps.tile([128, RC * Wp3], F32, tag="psa")
                        for t in range(25):
                            dy, dx = t // 5 - 2, t % 5 - 2
                            o = sb0 + dy * Wp3 + dx
                            nc.tensor.matmul(ps_a[:, :N],
                                             ws["pair_diag"][:, t * 128:(t + 1) * 128],
                                             v0t1[:, o:o + N],
                                             start=(t == 0), stop=(t == 24))
                        ps_b = cps.tile([128, RC * Wp3], F32, tag="psb")
                        for i, (dy, dxa, hasb) in enumerate(dw3_passes):
                            o = sb0 + dy * Wp3 + dxa
                            nc.tensor.matmul(ps_b[:, :N],
                                             ws["dw3_diag"][:, i * 128:(i + 1) * 128],
                                             v0t2[:, o:o + N],
                                             start=(i == 0), stop=(i == len(dw3_passes) - 1))

                        def inner(ap_flat, lo, hi):
                            # interior view of a PSUM chunk (starts at free 0)
                            return ap_flat[lo:hi, :N].rearrange(
                                "p (r w) -> p r w", w=Wp3)[:, :, 3:3 + W]

                        def inner_v0(ap_flat, lo, hi):
                            # interior view of the padded v0 buffers at this chunk
                            return ap_flat[lo:hi, sb0:sb0 + N].rearrange(
                                "p (r w) -> p r w", w=Wp3)[:, :, 3:3 + W]

                        ug_a = c2r.tile([128, RC * W], BF16, tag="uga")
                        ug_b = c2r.tile([128, RC * W], BF16, tag="ugb")
                        vb_a = c2r.tile([128, RC * W], BF16, tag="vba")
                        vb_b = c2r.tile([128, RC * W], BF16, tag="vbb")
                        g0v = vg0_hist.pop(ci)
                        nc.scalar.activation(ug_a[0:64, :NN], g0v[:, :NN], Act.Gelu,
                                             bias=ws["g0_bias"], scale=ws["g0_scale"])
                        nc.scalar.activation(
                            ug_a[64:128, :NN].rearrange("p (r w) -> p r w", w=W),
                            inner(ps_a, 0, 64), Act.Gelu, bias=ws["pair_bias"][0:64])
                        nc.scalar.activation(
                            ug_b[0:64, :NN].rearrange("p (r w) -> p r w", w=W),
                            inner(ps_a, 64, 128), Act.Gelu, bias=ws["pair_bias"][64:128])
                        nc.scalar.activation(
                            ug_b[64:128, :NN].rearrange("p (r w) -> p r w", w=W),
                            inner(ps_b, 0, 64), Act.Gelu, bias=ws["dw3_bias"])
                        nc.vector.tensor_scalar(out=vb_a[0:64, :NN], in0=g0v[:, :NN],
                                                scalar1=ws["s1a"][0:64],
                                                scalar2=ws["t1a"][0:64],
                                                op0=Alu.mult, op1=Alu.add)
                        nc.vector.tensor_scalar(out=vb_a[64:128, :NN],
                                                in0=inner_v0(v0t1, 0, 64),
                                                scalar1=ws["s1a"][64:128],
                                                scalar2=ws["t1a"][64:128],
                                                op0=Alu.mult, op1=Alu.add)
                        nc.vector.tensor_scalar(out=vb_b[0:64, :NN],
                                                in0=inner_v0(v0t1, 64, 128),
                                                scalar1=ws["s1b"][0:64],
                                                scalar2=ws["t1b"][0:64],
                                                op0=Alu.mult, op1=Alu.add)
                        nc.vector.tensor_scalar(out=vb_b[64:128, :NN],
                                                in0=inner_v0(v0t2, 0, 64),
                                                scalar1=ws["s1b"][64:128],
                                                scalar2=ws["t1b"][64:128],
                                                op0=Alu.mult, op1=Alu.add)
                        z1a = c2r.tile([128, RC * W], BF16, tag="z1a")
                        z1b = c2r.tile([128, RC * W], BF16, tag="z1b")
                        nc.vector.tensor_tensor(out=z1a[:, :NN], in0=ug_a[:, :NN],
                                                in1=vb_a[:, :NN], op=Alu.mult)
                        nc.vector.tensor_tensor(out=z1b[:, :NN], in0=ug_b[:, :NN],
                                                in1=vb_b[:, :NN], op=Alu.mult)
                        xpres = xp_hist.pop(ci)
                        for cg in range(2):
                            ops = cps.tile([128, RC * W], F32, tag="ops", bufs=2)
                            nc.tensor.matmul(ops[:, :NN],
                                             ws["fc2aT"][:, (cg * 2) * 128:(cg * 2 + 1) * 128],
                                             z1a[:, :NN], start=True, stop=False)
                            nc.tensor.matmul(ops[:, :NN],
                                             ws["fc2aT"][:, (cg * 2 + 1) * 128:(cg * 2 + 2) * 128],
                                             z1b[:, :NN], start=False, stop=False)
                            nc.tensor.matmul(ops[:, :NN],
                                             ws["fc2bT_g0"][:, cg * 128:(cg + 1) * 128],
                                             g0v[:, :NN], start=False, stop=False)
                            opsv = ops[:, :NN].rearrange("p (r w) -> p r w", w=W)
                            nc.tensor.matmul(opsv,
                                             ws["fc2bT_g12"][:, cg * 128:(cg + 1) * 128],
                                             inner_v0(v0t1, 0, 128), start=False, stop=False)
                            nc.tensor.matmul(opsv,
                                             ws["fc2bT_g3"][:, cg * 128:(cg + 1) * 128],
                                             inner_v0(v0t2, 0, 64), start=False, stop=True)
                            ob = c2r.tile([96, RC * W], F32, tag=f"ob{cg}", bufs=1)
                            nc.vector.tensor_scalar(out=ob[:, :NN], in0=ops[:96, :NN],
                                                    scalar1=ws["s3v"][:, cg:cg + 1],
                                                    scalar2=ws["out_bias"][:, cg:cg + 1],
                                                    op0=Alu.mult, op1=Alu.add)
                            oc = c2r.tile([96, RC * W], F32, tag=f"oc{cg}", bufs=1)
                            nc.vector.tensor_tensor(out=oc[:, :NN], in0=ob[:, :NN],
                                                    in1=xpres[cg][:, :NN], op=Alu.add)
                            nc.sync.dma_start(
                                out=out_t[cg * 96:(cg + 1) * 96, c0 * W:c0 * W + NN],
                                in_=oc[:, :NN])

                    for ci in range(NCH + 2):
                        if ci < NCH:
                            emit_c1(ci)
                        if ci >= 2:
                            emit_c2(ci - 2)
            _wpc_cm.__exit__(None, None, None)
    return out_t.name


# ----------------------------------------------------------------------------
# host entry
# ----------------------------------------------------------------------------

_CACHE = {}


def make_program(H, W, n_cores, attn_scale, dw3_passes):
    key = (H, W, n_cores, round(attn_scale, 9))
    if key in _CACHE:
        return _CACHE[key]
    nc = bacc.Bacc("TRN2", target_bir_lowering=False, debug=False, num_devices=n_cores)
    out_name = build(nc, H, W, n_cores, attn_scale, dw3_passes)
    nc.compile()
    _CACHE[key] = (nc, out_name)
    return nc, out_name


def make_in_maps(inputs):
    import ml_dtypes
    x = np.asarray(inputs["x"], np.float32)
    B = x.shape[0]
    wdict = _prep_weights({k: np.asarray(v) for k, v in inputs.items()})
    base = {}
    for k, (shp, d) in WSPEC.items():
        base["w_" + k] = wdict[k][0].reshape(shp)
    in_maps = []
    for b in range(B):
        m = dict(base)
        m["x"] = np.ascontiguousarray(x[b].reshape(-1, x.shape[-1]).T) \
            .astype(np.float16)
        in_maps.append(m)
    return in_maps, wdict


def kernel(**inputs):
    x = np.asarray(inputs["x"], np.float32)
    B, H, W, C = x.shape
    in_maps, wdict = make_in_maps(inputs)
    nc, out_name = make_program(H, W, B, wdict["_attn_scale"][0],
                                wdict["_dw3_passes"][0])
    res = bass_utils.run_bass_kernel_spmd(nc, in_maps, core_ids=list(range(B)))
    return np.stack([res.results[b][out_name].reshape(C, H, W).transpose(1, 2, 0)
                     for b in range(B)])


# revision 81
# speedup vs baseline: 1.0329x; 1.0329x over previous
"""Trainium2 Bass kernel for nn_Block_87351044866235 (sparse_attention).

Data-parallel over batch: 8 samples -> 8 NeuronCores. Channel-major
layout [C, H*W] on chip, everything fp16 on the PE (psum accumulation
stays f32). Depthwise convs run as diagonal matmuls with weights padded
to 128 columns (FWL); the qkv 1x1 is composed into its 3x3 depthwise
(27 full-matrix taps applied to yn directly); the +x residual is folded
into the pos conv's center tap. q/k are kept in plain fp16 (no hi/lo
split - fp16 logit error ~1e-3 vs ~0.07 rank gaps), transposed on the
PE, and reduced to per-head gram matrices. yn1 (padded layout), yn2 and
v stay SBUF-resident; the attn-out/proj/LN2 stage is fused into the FFN
chunk pipeline (c1) so the PE never drains between phases. LN stats use
an exactly-representable 2^-8 ones weight with 4/3 corrections folded
into existing ops. The dynamic-k gate runs at the start of phase B (one
sigmoid table load; scalar in A never leaves the sqrt table) and its
batch mean AllReduce overlaps the qkv conv + gram.
"""
import sys, os

for _p in ("/opt/trn_rl_repo", "/root/.axon_site/_ro/trn_rl_repo"):
    if os.path.isdir(_p) and _p not in sys.path:
        sys.path.append(_p)

import numpy as np
import concourse.bass as bass
import concourse.bacc as bacc
import concourse.tile as tile
from concourse import mybir
from concourse import bass_utils

try:
    from concourse import tile_utils as _tu
    _tu.max_sbuf_usage = 208 * 1024
except Exception:
    pass

dt = mybir.dt
Alu = mybir.AluOpType
Act = mybir.ActivationFunctionType
AX = mybir.AxisListType.X

EMBED, PDIM, HEADS, HID = 192, 96, 8, 256
CPH = PDIM // HEADS  # 12
SLOP = 8
RC = 3    # conv output rows per chunk
BR = 12   # rows per band (phase A x loads)

# BF16 historically named; now fp16 (same PE/DVE throughput, 11-bit mantissa)
F32, F32R, BF16 = dt.float32, dt.float32r, dt.float16


def _ceil(a, b):
    return (a + b - 1) // b


# ----------------------------------------------------------------------------
# host-side weight prep: everything 2D [partitions, free]
# ----------------------------------------------------------------------------

def _prep_weights(p):
    import ml_dtypes
    w = {}
    f32r = lambda a: (np.ascontiguousarray(a, np.float32), F32R)
    f32 = lambda a: (np.ascontiguousarray(a, np.float32), F32)
    bf16 = lambda a: (np.ascontiguousarray(a, np.float32)
                      .astype(np.float16), BF16)
    eps_bn = 1e-5

    w["ident"] = f32(np.eye(128, dtype=np.float32))

    # pos depthwise diag: [96, (t*2+cg)*128] (bf16, 128-col blocks for FWL)
    pw = p["pos_w"][:, 0]  # [192,3,3]
    pos_d = np.zeros((96, 18 * 128), np.float32)
    for t in range(9):
        dy, dx = t // 3 - 1, t % 3 - 1
        for cg in range(2):
            blk = np.diag(pw[cg * 96:(cg + 1) * 96, dy + 1, dx + 1])
            if t == 4:
                # residual +x folded into the center tap (1.0 is exact in bf16)
                blk = blk + np.eye(96, dtype=np.float32)
            pos_d[:, (t * 2 + cg) * 128:(t * 2 + cg) * 128 + 96] = blk
    w["pos_diag"] = bf16(pos_d)
    w["pos_b"] = f32(p["pos_b"].reshape(2, 96).T)  # [96, 2]

    g1v, b1v = p["ln1_g"], p["ln1_b"]
    qw = p["qkv_w"][:, :, 0, 0]  # [288, 96]
    qw_eff = qw * g1v[None, :96]
    qdw = p["qkv_dw_w"][:, 0]    # [288,3,3]
    # composed qkv conv: tap (t, j) stationary = (diag(qdw_tj) @ qw_eff_j).T
    # bias would be (sum_t qdw_tj) * (qw_j @ ln1_b); exact only with
    # zero-padded yn when that bias is 0 -- ln1_b is zeros in setup_inputs.
    qb = qw @ b1v[:96]
    assert np.abs(qb).max() < 1e-12, "composed qkv conv assumes ln1_b == 0"
    qdwc = np.zeros((96, 27 * 128), np.float32)
    for t in range(9):
        dy, dx = t // 3 - 1, t % 3 - 1
        for j in range(3):
            M = qw_eff[j * 96:(j + 1) * 96, :] \
                * qdw[j * 96:(j + 1) * 96, dy + 1, dx + 1][:, None]  # [out, in]
            qdwc[:, (t * 3 + j) * 128:(t * 3 + j) * 128 + 96] = M.T
    w["qdwc"] = bf16(qdwc)

    gw1 = p["gate_w1"][:, :, 0, 0]  # [96, 192]
    gw1_eff = gw1 * g1v[None, :]
    gw1p = np.zeros((96, 2 * 128), np.float32)
    for cg in range(2):
        gw1p[:, cg * 128:cg * 128 + 96] = gw1_eff[:, cg * 96:(cg + 1) * 96].T
    w["gate_w1T"] = bf16(gw1p)
    w["gate_b1"] = f32((p["gate_b1"] + gw1 @ b1v).reshape(96, 1))
    gw2p = np.zeros((96, 128), np.float32)
    gw2p[:, 0] = p["gate_w2"][:, :, 0, 0].reshape(96)
    w["gate_w2T"] = bf16(gw2p)
    w["gate_b2"] = f32(p["gate_b2"].reshape(1, 1))

    attn_scale = float(p["attn1"][0] + p["attn2"][0] + p["attn3"][0]
                       + p["attn4"][0])
    pj = p["proj_w"][:, :, 0, 0]
    pj1 = pj[:, :96] * attn_scale     # attn out scale folded into proj1
    pj2 = pj[:, 96:] * g1v[None, 96:]
    pj1p = np.zeros((96, 2 * 128), np.float32)
    pj2p = np.zeros((96, 2 * 128), np.float32)
    for cg in range(2):
        pj1p[:, cg * 128:cg * 128 + 96] = pj1[cg * 96:(cg + 1) * 96].T
        pj2p[:, cg * 128:cg * 128 + 96] = pj2[cg * 96:(cg + 1) * 96].T
    w["proj1T"] = bf16(pj1p)
    w["proj2T"] = bf16(pj2p)
    w["proj_bias"] = f32((pj[:, 96:] @ b1v[96:]).reshape(2, 96).T)  # [96, 2]

    w["_attn_scale"] = (attn_scale, None)
    w["tempvec"] = f32(np.repeat(p["temperature"].reshape(HEADS), CPH).reshape(96, 1))

    g2v, b2v = p["ln2_g"], p["ln2_b"]
    f1 = p["fc1_w"][:, :, 0, 0]  # [256, 192]
    f1_eff = f1 * g2v[None, :]
    fc1 = np.zeros((96, 4 * 128), np.float32)
    for mg in range(2):
        for cg in range(2):
            fc1[:, (mg * 2 + cg) * 128:(mg * 2 + cg + 1) * 128] = \
                f1_eff[mg * 128:(mg + 1) * 128, cg * 96:(cg + 1) * 96].T
    w["fc1T"] = bf16(fc1)
    w["fc1_bias"] = f32((f1 @ b2v).reshape(2, 128).T)  # [128, 2]

    s1 = p["bn1_g"] / np.sqrt(p["bn1_v"] + eps_bn)
    t1 = p["bn1_b"] - p["bn1_m"] * s1
    s2 = p["bn2_g"] / np.sqrt(p["bn2_v"] + eps_bn)
    t2 = p["bn2_b"] - p["bn2_m"] * s2
    s3 = p["bn3_g"] / np.sqrt(p["bn3_v"] + eps_bn)
    t3 = p["bn3_b"] - p["bn3_m"] * s3

    dw1w, dw2w, dw3w = p["dw1_w"][:, 0], p["dw2_w"][:, 0], p["dw3_w"][:, 0]
    dw1b, dw2b, dw3b = p["dw1_b"], p["dw2_b"], p["dw3_b"]
    s1g = [s1[i * 64:(i + 1) * 64] for i in range(4)]
    t1g = [t1[i * 64:(i + 1) * 64] for i in range(4)]

    pair_d = np.zeros((128, 25 * 128), np.float32)
    for t in range(25):
        dy, dx = t // 5 - 2, t % 5 - 2
        blk = np.zeros((128, 128), np.float32)
        d2 = dw2w[:, dy + 2, dx + 2] * s1g[2]
        if dy == 0 and dx == 0:
            d2 = d2 + s1g[2]
        blk[64:, 64:] = np.diag(d2)
        if -1 <= dy <= 1 and -1 <= dx <= 1:
            d1 = dw1w[:, dy + 1, dx + 1] * s1g[1]
            if dy == 0 and dx == 0:
                d1 = d1 + s1g[1]
            blk[:64, :64] = np.diag(d1)
        pair_d[:, t * 128:(t + 1) * 128] = blk
    w["pair_diag"] = bf16(pair_d)
    bc1 = t1g[1] * dw1w.sum((1, 2)) + dw1b + t1g[1]
    bc2 = t1g[2] * dw2w.sum((1, 2)) + dw2b + t1g[2]
    w["pair_bias"] = f32(np.concatenate([bc1, bc2]).reshape(128, 1))

    # rows 64:128 of v0t2 hold the same data stored shifted +1, so a read at
    # AP offset (dy, dxa) yields tap (dy, dxa-1) for those rows.
    dw3_passes = []
    for dy in range(-3, 4):
        for dxa in (-2, 0, 2):
            dw3_passes.append((dy, dxa, True))
        dw3_passes.append((dy, 3, False))
    dw3_d = np.zeros((128, len(dw3_passes) * 128), np.float32)
    for i, (dy, dxa, hasb) in enumerate(dw3_passes):
        wa = dw3w[:, dy + 3, dxa + 3] * s1g[3]
        if dy == 0 and dxa == 0:
            wa = wa + s1g[3]
        dw3_d[:64, i * 128:i * 128 + 64] = np.diag(wa)
        if hasb:
            wb = dw3w[:, dy + 3, dxa - 1 + 3] * s1g[3]
            if dy == 0 and dxa - 1 == 0:
                wb = wb + s1g[3]
            dw3_d[64:, i * 128:i * 128 + 64] = np.diag(wb)
    w["dw3_diag"] = bf16(dw3_d)
    w["_dw3_passes"] = (dw3_passes, None)
    w["dw3_bias"] = f32((t1g[3] * dw3w.sum((1, 2)) + dw3b + t1g[3]).reshape(64, 1))

    d0w, d0b = p["dw0_w"][:, 0, 0, 0], p["dw0_b"]
    w["g0_scale"] = f32(((d0w + 1.0) * s1g[0]).reshape(64, 1))
    w["g0_bias"] = f32(((d0w + 1.0) * t1g[0] + d0b).reshape(64, 1))

    f2 = p["fc2_w"][:, :, 0, 0]  # [192, 256]
    f2a = f2 * s2[None, :]
    fc2a = np.zeros((128, 4 * 128), np.float32)
    for cg in range(2):
        for kg in range(2):
            fc2a[:, (cg * 2 + kg) * 128:(cg * 2 + kg) * 128 + 96] = \
                f2a[cg * 96:(cg + 1) * 96, kg * 128:(kg + 1) * 128].T
    w["fc2aT"] = bf16(fc2a)
    # fc2's t2-term applied to vb (= bn1'd v0) directly: f2*t2*s1*v0raw +
    # f2*t2*t1 == f2*t2*vb, so two FLAT matmuls on vb_a/vb_b replace the
    # g0/g12/g3 passes (and the cstv constant folds into vb)
    f2bv = f2 * t2[None, :]
    fbva = np.zeros((128, 2 * 128), np.float32)
    fbvb = np.zeros((128, 2 * 128), np.float32)
    for cg in range(2):
        fbva[:, cg * 128:cg * 128 + 96] = f2bv[cg * 96:(cg + 1) * 96, 0:128].T
        fbvb[:, cg * 128:cg * 128 + 96] = f2bv[cg * 96:(cg + 1) * 96, 128:256].T
    w["fc2bvA"] = bf16(fbva)
    w["fc2bvB"] = bf16(fbvb)
    w["s3v"] = f32(np.stack([s3[:96], s3[96:]], axis=1))          # [96, 2]
    w["out_bias"] = f32(np.stack([t3[:96], t3[96:]], axis=1))     # [96, 2]

    sg = np.where(s1 == 0, 1.0, s1)
    padv = -t1 / sg
    w["padv1"] = f32(np.concatenate([padv[64:128], padv[128:192]]).reshape(128, 1))
    w["padv2"] = f32(np.concatenate([padv[192:256], padv[192:256]]).reshape(128, 1))
    w["s1a"] = f32(s1[:128].reshape(128, 1))
    w["s1b"] = f32(s1[128:].reshape(128, 1))
    w["t1a"] = f32(t1[:128].reshape(128, 1))
    w["t1b"] = f32(t1[128:].reshape(128, 1))

    # bf16 stats weights: exactly-representable 2^-8; the (EMBED/256)
    # correction factor 4/3 is applied in the finishing ops on-chip
    ones = np.full((96, 128), 2.0 ** -8, np.float32)
    w["ones_st"] = bf16(ones)
    w["ones_st2"] = bf16(ones)
    w["epsv"] = f32(np.full((128, 1), 1e-6, np.float32))
    w["epsv2"] = f32(np.full((128, 1), 1e-6, np.float32))
    vm = np.zeros((96, 96), np.float32)
    for h in range(HEADS):
        vm[h * CPH:(h + 1) * CPH, h * CPH:(h + 1) * CPH] = 1.0
    w["vmask"] = f32(vm)
    w["vmaskb"] = bf16((vm - 1.0) * 60.0)
    return w


WSPEC = {
    "ident": ([128, 128], F32), "pos_diag": ([96, 18 * 128], BF16),
    "pos_b": ([96, 2], F32), "qdwc": ([96, 27 * 128], BF16),
    "gate_w1T": ([96, 2 * 128], BF16), "gate_b1": ([96, 1], F32),
    "gate_w2T": ([96, 128], BF16), "gate_b2": ([1, 1], F32),
    "proj1T": ([96, 2 * 128], BF16), "proj2T": ([96, 2 * 128], BF16),
    "proj_bias": ([96, 2], F32), "tempvec": ([96, 1], F32),
    "fc1T": ([96, 4 * 128], BF16), "fc1_bias": ([128, 2], F32),
    "pair_diag": ([128, 25 * 128], BF16), "pair_bias": ([128, 1], F32),
    "dw3_diag": ([128, 28 * 128], BF16), "dw3_bias": ([64, 1], F32),
    "g0_scale": ([64, 1], F32), "g0_bias": ([64, 1], F32),
    "fc2aT": ([128, 4 * 128], BF16), "fc2bvA": ([128, 2 * 128], BF16),
    "fc2bvB": ([128, 2 * 128], BF16),
    "s3v": ([96, 2], F32), "out_bias": ([96, 2], F32),
    "padv1": ([128, 1], F32),
    "padv2": ([128, 1], F32),
    "s1a": ([128, 1], F32), "s1b": ([128, 1], F32),
    "t1a": ([128, 1], F32), "t1b": ([128, 1], F32),
    "ones_st": ([96, 128], BF16), "ones_st2": ([96, 128], BF16),
    "epsv": ([128, 1], F32), "epsv2": ([128, 1], F32),
    "vmask": ([96, 96], F32), "vmaskb": ([96, 96], BF16),
}

# weights only used by the fused attn-out/FFN phase (loaded during phase A)
C_ONLY_W = ['proj1T', 'proj2T', 'proj_bias', 'fc1T', 'fc1_bias', 'pair_diag',
            'pair_bias', 'dw3_diag', 'dw3_bias', 'g0_scale', 'g0_bias',
            'fc2aT', 'fc2bvA', 'fc2bvB', 's3v', 'out_bias',
            's1a', 's1b', 't1a', 't1b', 'padv1', 'padv2', 'ones_st2', 'epsv2']


# ----------------------------------------------------------------------------
# device kernel
# ----------------------------------------------------------------------------

def build(nc, H, W, n_cores, attn_scale, dw3_passes):
    S = H * W
    Wp1 = W + 2
    P1B = (BR + 2) * Wp1 + 2 * SLOP   # phase-A x band buffer (pad1)
    YB = 8                            # guard cells before yn1p row 0
    P1Y = YB + (H + 3) * Wp1          # full padded yn1 image (+top/bot pad rows)
    Wp3, Hp3 = W + 6, H + 6
    P3 = Hp3 * Wp3 + 2 * SLOP
    NCH = _ceil(H, RC)
    NB = _ceil(H, BR)

    x_t = nc.dram_tensor("x", [EMBED, S], BF16, kind="ExternalInput")
    out_t = nc.dram_tensor("out", [EMBED, S], F32, kind="ExternalOutput")
    wt = {k: nc.dram_tensor("w_" + k, shp, d, kind="ExternalInput")
          for k, (shp, d) in WSPEC.items()}

    def pd3(r):
        return SLOP + r * Wp3

    with tile.TileContext(nc) as tc:
        with (
            tc.tile_pool(name="dram", bufs=1, space="DRAM") as dram,
            tc.tile_pool(name="persist", bufs=1) as pers,
        ):
            ws = {}

            def _load_w(pool, names):
                for k in names:
                    shp, d = WSPEC[k]
                    tl = pool.tile(shp, d, tag="w_" + k, name="w_" + k)
                    nc.sync.dma_start(out=tl[:], in_=wt[k][:])
                    ws[k] = tl

            xc_sp = [dram.tile([96, S], BF16, name=f"xc_sp{i}") for i in range(2)]
            cc_in = dram.tile([1, 1], F32)
            cc_out = dram.tile([1, 1], F32)

            gsum = pers.tile([1, 64], F32)
            nc.vector.memset(gsum[:], 0.0)
            # per-chunk q/k squared-norm partials (cols 0:64 q, 64:128 k)
            nrm2a = pers.tile([96, 128], F32)
            dynk = pers.tile([96, 1], F32)
            probsT = pers.tile([96, 128], BF16)
            identb = pers.tile([96, 96], BF16)
            yn2res = pers.tile([96, S], BF16)   # LN1 out, ch 96:192 (flat)
            v_res = pers.tile([96, S], BF16)    # attn v image (flat)

            # phase-C weight pool opens first (released last, LIFO); its
            # DMAs are emitted at the end of phase A
            _wpc_cm = tc.tile_pool(name="wpC", bufs=1)
            wpc = _wpc_cm.__enter__()
            _wpab_cm = tc.tile_pool(name="wpAB", bufs=1)
            wpab = _wpab_cm.__enter__()
            _ab_names = [k for k in WSPEC if k not in C_ONLY_W]
            # pos weights first so the first conv matmul isn't DMA-gated
            _ab_names.sort(key=lambda k: 0 if k in ("pos_diag", "pos_b", "ones_st",
                                                    "epsv") else 1)
            _load_w(wpab, _ab_names)
            # padded resident yn1 image (LN1 out ch 0:96), bf16; its memset
            # is deferred into band 0 so it doesn't head the vector queue
            # and delay the first x DMA
            _ynp_cm = tc.tile_pool(name="ynp", bufs=1)
            ynp = _ynp_cm.__enter__()
            yn1p = ynp.tile([96, P1Y], BF16)

            # ================= PHASE A =================
            ci_g = 0
            with (
                tc.tile_pool(name="pa_band", bufs=2) as pab,
                tc.tile_pool(name="pa_rot", bufs=3) as par,
                tc.tile_pool(name="pa_ps", bufs=2, space="PSUM") as paps,
                tc.tile_pool(name="pa_psg", bufs=2, space="PSUM") as papsg,
            ):
                for b in range(NB):
                    r0, r1 = b * BR, min((b + 1) * BR, H)
                    ylo, yhi = max(r0 - 1, 0), min(r1 + 1, H)
                    xband = [pab.tile([96, P1B], BF16, tag=f"xb{cg}", name=f"xb{cg}")
                             for cg in range(2)]
                    for cg in range(2):
                        if b < 2:
                            nc.vector.memset(xband[cg][:].bitcast(F32), 0.0)
                        elif b == NB - 1 and yhi - (r0 - 1) < BR + 2:
                            o = SLOP + (yhi - (r0 - 1)) * Wp1
                            nc.vector.memset(xband[cg][:, o:].bitcast(F32), 0.0)
                        dst = xband[cg][:, SLOP + (ylo - (r0 - 1)) * Wp1:
                                        SLOP + (yhi - (r0 - 1)) * Wp1] \
                            .rearrange("p (r w) -> p r w", w=Wp1)[:, :, 1:1 + W]
                        nc.sync.dma_start(
                            out=dst,
                            in_=x_t[cg * 96:(cg + 1) * 96, ylo * W:yhi * W]
                            .rearrange("c (r w) -> c r w", w=W))
                    if b == 0:
                        # deferred init work overlaps the band-0 x DMA
                        nc.vector.memset(yn1p[:].bitcast(F32), 0.0)
                        nc.vector.memset(probsT[:].bitcast(F32), 0.0)
                        nc.vector.memset(nrm2a[:], 0.0)
                        nc.vector.tensor_copy(out=identb[:],
                                              in_=ws["ident"][:96, :96])
                    for c0 in range(r0, r1, RC):
                        nr_c = min(RC, H - c0)
                        N = nr_c * Wp1
                        NN = nr_c * W
                        sb0 = SLOP + (c0 - r0 + 1) * Wp1
                        xc_ch = [par.tile([96, RC * W], BF16, tag=f"xc{cg}",
                                          name=f"xc{cg}") for cg in range(2)]
                        xsq = [par.tile([96, RC * W], BF16, tag=f"xq{cg}",
                                        name=f"xq{cg}") for cg in range(2)]
                        for cg in range(2):
                            ps = paps.tile([128, RC * Wp1], F32, tag="posps", bufs=3)
                            for t in range(9):
                                dy, dx = t // 3 - 1, t % 3 - 1
                                o = sb0 + dy * Wp1 + dx
                                nc.tensor.matmul(
                                    ps[:, :N],
                                    ws["pos_diag"][:, (t * 2 + cg) * 128:(t * 2 + cg + 1) * 128],
                                    xband[cg][:, o:o + N],
                                    start=(t == 0), stop=(t == 8))
                            ps_int = ps[:96, :N].rearrange("p (r w) -> p r w", w=Wp1)[:, :, 1:1 + W]
                            xcv = xc_ch[cg][:, :NN].rearrange("p (r w) -> p r w", w=W)
                            nc.scalar.activation(xcv, ps_int, Act.Identity,
                                                 bias=ws["pos_b"][:, cg:cg + 1])
                            nc.scalar.square(xsq[cg][:, :NN], xc_ch[cg][:, :NN])
                            nc.sync.dma_start(out=xc_sp[cg][:, c0 * W:c0 * W + NN],
                                              in_=xc_ch[cg][:, :NN])
                        mu_ps = paps.tile([128, RC * W], F32, tag="mups")
                        m2_ps = paps.tile([128, RC * W], F32, tag="m2ps")
                        for cg in range(2):
                            nc.tensor.matmul(mu_ps[:, :NN], ws["ones_st"], xc_ch[cg][:, :NN],
                                             start=(cg == 0), stop=(cg == 1))
                            nc.tensor.matmul(m2_ps[:, :NN], ws["ones_st"], xsq[cg][:, :NN],
                                             start=(cg == 0), stop=(cg == 1))
                        musq = par.tile([128, RC * W], F32, tag="musq")
                        nc.scalar.activation(musq[:, :NN], mu_ps[:, :NN], Act.Square,
                                             scale=4.0 / 3.0)
                        var = par.tile([128, RC * W], F32, tag="var")
                        nc.vector.scalar_tensor_tensor(
                            out=var[:, :NN], in0=m2_ps[:, :NN], scalar=4.0 / 3.0,
                            in1=musq[:, :NN], op0=Alu.mult, op1=Alu.subtract)
                        sd = par.tile([128, RC * W], F32, tag="sd")
                        nc.scalar.activation(sd[:, :NN], var[:, :NN], Act.Sqrt, bias=ws["epsv"])
                        rstd = par.tile([128, RC * W], F32, tag="rstd")
                        nc.vector.reciprocal_approx_fast(out=rstd[:, :NN], in_=sd[:, :NN])
                        # cg0: normalize straight into the padded resident yn1
                        td0 = par.tile([96, RC * W], F32, tag="td0")
                        nc.vector.scalar_tensor_tensor(
                            out=td0[:, :NN], in0=mu_ps[:96, :NN], scalar=-4.0 / 3.0,
                            in1=xc_ch[0][:, :NN], op0=Alu.mult, op1=Alu.add)
                        yb0 = YB + (c0 + 1) * Wp1 + 1
                        nc.vector.tensor_tensor(
                            out=yn1p[:, yb0:yb0 + nr_c * Wp1]
                            .rearrange("p (r w) -> p r w", w=Wp1)[:, :, 0:W],
                            in0=td0[:, :NN].rearrange("p (r w) -> p r w", w=W),
                            in1=rstd[:96, :NN].rearrange("p (r w) -> p r w", w=W),
                            op=Alu.mult)
                        # cg1: straight into the flat resident yn2 image
                        td1 = par.tile([96, RC * W], F32, tag="td1")
                        nc.vector.scalar_tensor_tensor(
                            out=td1[:, :NN], in0=mu_ps[:96, :NN], scalar=-4.0 / 3.0,
                            in1=xc_ch[1][:, :NN], op0=Alu.mult, op1=Alu.add)
                        nc.vector.tensor_tensor(out=yn2res[:, c0 * W:c0 * W + NN],
                                                in0=td1[:, :NN],
                                                in1=rstd[:96, :NN], op=Alu.mult)

                # phase-C weight DMAs overlap phase B compute
                _load_w(wpc, C_ONLY_W)

            # ================= PHASE B =================
            # composed qkv 3x3 conv (27 full-matrix taps) straight off yn1p;
            # hi/lo bf16 split; DMA-xbar transposes into qkband; gram matmuls.
            with (
                tc.tile_pool(name="pb_rot", bufs=3) as pbr,
                tc.tile_pool(name="gram_ps", bufs=1, space="PSUM") as gpsp,
            ):
                g1_ps = gpsp.tile([96, 96], F32)
                with (
                    tc.tile_pool(name="pb_ps", bufs=2, space="PSUM") as pbps,
                    tc.tile_pool(name="pb_pst", bufs=2, space="PSUM") as pbpst,
                ):
                    # dynamic-k gate (here instead of phase A so the scalar
                    # engine never swaps between the sqrt and sigmoid tables)
                    for ci in range(NCH):
                        c0 = ci * RC
                        nr_c = min(RC, H - c0)
                        NN = nr_c * W
                        gps = pbps.tile([128, RC * W], F32, tag="gps", bufs=2)
                        for rr in range(nr_c):
                            yb0 = YB + (c0 + rr + 1) * Wp1 + 1
                            nc.tensor.matmul(gps[:, rr * W:(rr + 1) * W],
                                             ws["gate_w1T"][:, 0:128],
                                             yn1p[:, yb0:yb0 + W],
                                             start=True, stop=False,
                                             skip_group_check=True)
                        nc.tensor.matmul(gps[:, :NN], ws["gate_w1T"][:, 128:256],
                                         yn2res[:, c0 * W:c0 * W + NN],
                                         start=False, stop=True,
                                         skip_group_check=True)
                        g1s = pbr.tile([96, RC * W], BF16, tag="g1s")
                        nc.scalar.activation(g1s[:, :NN], gps[:96, :NN], Act.Relu,
                                             bias=ws["gate_b1"])
                        g2ps = pbps.tile([128, RC * W], F32, tag="gps", bufs=2)
                        nc.tensor.matmul(g2ps[:, :NN], ws["gate_w2T"], g1s[:, :NN],
                                         start=True, stop=True)
                        sgt = pbr.tile([1, RC * W], F32, tag="sgt")
                        nc.scalar.activation(sgt[:, :NN], g2ps[:1, :NN], Act.Sigmoid,
                                             bias=ws["gate_b2"],
                                             accum_out=gsum[0:1, ci:ci + 1])
                    # gate mean -> AllReduce -> dynk (overlaps the conv + gram)
                    gred = pers.tile([1, 1], F32)
                    nc.vector.reduce_sum(gred[:], gsum[0:1, 0:NCH], axis=AX)
                    gsc = pers.tile([1, 1], F32)
                    nc.vector.tensor_scalar_mul(gsc[:], gred[:],
                                                float(CPH) / (n_cores * S))
                    nc.sync.dma_start(out=cc_in[:], in_=gsc[:])
                    nc.gpsimd.collective_compute(
                        "AllReduce", Alu.add,
                        replica_groups=[list(range(n_cores))],
                        ins=[cc_in.opt()], outs=[cc_out.opt()])
                    nc.sync.dma_start(out=dynk[:],
                                      in_=cc_out[:].partition_broadcast(96))
                    for ci in range(NCH):
                        c0 = ci * RC
                        nr_c = min(RC, H - c0)
                        N = nr_c * Wp1
                        NN = nr_c * W
                        sb0 = YB + (c0 + 1) * Wp1
                        qkf = {}
                        for j in range(3):
                            ps = pbps.tile([128, RC * Wp1], F32, tag="dwps", bufs=3)
                            for t in range(9):
                                dy, dx = t // 3 - 1, t % 3 - 1
                                o = sb0 + dy * Wp1 + dx
                                nc.tensor.matmul(
                                    ps[:, :N],
                                    ws["qdwc"][:, (t * 3 + j) * 128:(t * 3 + j + 1) * 128],
                                    yn1p[:, o:o + N],
                                    start=(t == 0), stop=(t == 8))
                            ps_int = ps[:96, :N].rearrange("p (r w) -> p r w", w=Wp1)[:, :, 1:1 + W]
                            if j == 2:
                                nc.scalar.copy(
                                    v_res[:, c0 * W:c0 * W + NN]
                                    .rearrange("p (r w) -> p r w", w=W), ps_int)
                            else:
                                qf = pbr.tile([96, RC * W], BF16, tag=f"qf{j}")
                                nc.scalar.copy(
                                    qf[:, :NN].rearrange("p (r w) -> p r w", w=W),
                                    ps_int)
                                qkf[j] = qf
                                # ||q||^2 / ||k||^2 partials on the idle B
                                # scalar engine (replaces the qq/kk grams)
                                sqs = pbr.tile([96, RC * W], BF16, tag="sqs")
                                nc.scalar.activation(
                                    sqs[:, :NN], qf[:, :NN], Act.Square,
                                    accum_out=nrm2a[:, j * 64 + ci:j * 64 + ci + 1])
                        qkb = pbr.tile([128, RC * 192], BF16, tag="qkb", bufs=3)
                        for rr in range(nr_c):
                            rl = rr * W
                            ro = rr * 192
                            tps = pbpst.tile([W, 192], BF16, tag="qktp")
                            for idx in range(2):
                                nc.tensor.transpose(
                                    tps[:, idx * 96:(idx + 1) * 96],
                                    qkf[idx][:, rl:rl + W], identb[:])
                            nc.vector.tensor_copy(out=qkb[:, ro:ro + 192],
                                                  in_=tps[:])
                        for rr in range(nr_c):
                            ro = rr * 192
                            first, last = (c0 + rr == 0), (c0 + rr == H - 1)
                            nc.tensor.matmul(g1_ps[:], qkb[:, ro:ro + 96],
                                             qkb[:, ro + 96:ro + 192],
                                             start=first, stop=last)

                # ---- attn block ----
                ident = ws["ident"]
                with (
                    tc.tile_pool(name="at_ps", bufs=2, space="PSUM") as atps,
                    tc.tile_pool(name="at_sb", bufs=1) as ab,
                ):
                    g1sb = ab.tile([96, 96], F32)
                    nc.scalar.copy(g1sb[:], g1_ps[:])
                    nq2 = ab.tile([96, 1], F32)
                    nc.vector.reduce_sum(nq2[:], nrm2a[:, 0:NCH], axis=AX)
                    nk2 = ab.tile([96, 1], F32)
                    nc.vector.reduce_sum(nk2[:], nrm2a[:, 64:64 + NCH], axis=AX)

                    def rsqrt_clamped(nm, src):
                        sq = ab.tile([96, 1], F32, tag=nm + "sq")
                        nc.scalar.sqrt(sq[:], src[:])
                        cl = ab.tile([96, 1], F32, tag=nm + "cl")
                        nc.vector.tensor_scalar_max(cl[:], sq[:], 1e-12)
                        rvv = ab.tile([96, 1], F32, tag=nm)
                        nc.vector.reciprocal(rvv[:], cl[:])
                        return rvv

                    rq = rsqrt_clamped("rq", nq2)
                    rk = rsqrt_clamped("rk", nk2)
                    rqt = ab.tile([96, 1], F32)
                    nc.vector.tensor_tensor(out=rqt[:], in0=rq[:], in1=ws["tempvec"][:],
                                            op=Alu.mult)
                    asr = ab.tile([96, 96], F32)
                    nc.vector.tensor_scalar_mul(asr[:], g1sb[:], rqt[:])
                    as_ps = atps.tile([96, 96], F32, tag="atp")
                    nc.tensor.transpose(as_ps[:], asr[:], ident[:96, :96])
                    ast = ab.tile([96, 96], F32)
                    nc.vector.tensor_scalar_mul(ast[:], as_ps[:], rk[:])
                    as2_ps = atps.tile([96, 96], F32, tag="atp")
                    nc.tensor.transpose(as2_ps[:], ast[:], ident[:96, :96])
                    as2 = ab.tile([96, 96], F32)
                    nc.scalar.copy(as2[:], as2_ps[:])
                    # mask off-head-block entries to -60 on UNSHIFTED logits
                    # (fp16 ranks need full resolution near zero)
                    t0m = ab.tile([96, 96], BF16)
                    nc.vector.tensor_tensor(out=t0m[:], in0=as2[:], in1=ws["vmask"][:],
                                            op=Alu.mult)
                    amf = ab.tile([96, 96], BF16)
                    nc.vector.tensor_tensor(out=amf[:], in0=t0m[:], in1=ws["vmaskb"][:],
                                            op=Alu.add)
                    # rank+1 over full row via pairwise is_ge (16-bit: 2x DVE)
                    rnk3 = ab.tile([96, 96 * 96], BF16)
                    a_i = amf[:].unsqueeze(1).broadcast_to([96, 96, 96])
                    a_d = amf[:].unsqueeze(2).broadcast_to([96, 96, 96])
                    rvw = rnk3[:].rearrange("p (i d) -> p i d", d=96)
                    nc.vector.tensor_tensor(out=rvw, in0=a_i, in1=a_d, op=Alu.is_ge)
                    rank1 = ab.tile([96, 96], F32)
                    nc.vector.reduce_sum(rank1[:].unsqueeze(2), rvw, axis=AX)
                    sel = ab.tile([96, 96], F32)
                    nc.vector.tensor_tensor(out=sel[:], in0=rank1[:],
                                            in1=dynk[:].broadcast_to([96, 96]), op=Alu.is_le)
                    t60 = ab.tile([96, 96], F32)
                    nc.vector.tensor_scalar_add(t60[:], amf[:], 60.0)
                    am = ab.tile([96, 96], F32)
                    nc.vector.tensor_tensor(out=am[:], in0=t60[:], in1=sel[:], op=Alu.mult)
                    nc.vector.tensor_scalar_add(am[:], am[:], -60.0)
                    mx = ab.tile([96, 1], F32)
                    nc.vector.reduce_max(mx[:], am[:], axis=AX)
                    nmx = ab.tile([96, 1], F32)
                    nc.vector.tensor_scalar_mul(nmx[:], mx[:], -1.0)
                    ex = ab.tile([96, 96], F32)
                    nc.scalar.activation(ex[:], am[:], Act.Exp, bias=nmx[:])
                    sme = ab.tile([96, 1], F32)
                    nc.vector.reduce_sum(sme[:], ex[:], axis=AX)
                    rsm = ab.tile([96, 1], F32)
                    nc.vector.reciprocal(rsm[:], sme[:])
                    probs = ab.tile([96, 96], F32)
                    nc.vector.tensor_scalar_mul(probs[:], ex[:], rsm[:])
                    pt_ps = atps.tile([96, 96], F32, tag="atp2")
                    nc.tensor.transpose(pt_ps[:], probs[:], ident[:96, :96])
                    nc.scalar.copy(probsT[:, 0:96], pt_ps[:])

            _ynp_cm.__exit__(None, None, None)
            _wpab_cm.__exit__(None, None, None)

            # ========== PHASE C (attn-out/proj/LN2 fused with FFN pipeline) ==
            with tc.tile_pool(name="c_v0", bufs=1) as cv0:
                v0t1 = cv0.tile([128, P3], BF16)
                v0t2 = cv0.tile([128, P3], BF16)
                with (
                    tc.tile_pool(name="c1_rot", bufs=2) as c1r,
                    tc.tile_pool(name="c2_rot", bufs=2) as c2r,
                    tc.tile_pool(name="c_ps", bufs=1, space="PSUM") as cps,
                ):
                    # pad cells must hold -t1/s1 so the bn-folded depthwise
                    # reads zeros in v0_bn space at image borders; only the
                    # border cells need init (interior is overwritten below)
                    head_end = pd3(3) + 4
                    tail_beg = pd3(H + 3) - 3
                    mid0 = pd3(3) + W + 3
                    for v0t, padv in ((v0t1, "padv1"), (v0t2, "padv2")):
                        # memset must run on f32 views; for bf16 tiles use
                        # even-aligned (wider) regions -- the extra cells are
                        # data cells the fc1 writes below overwrite anyway
                        f32v = v0t[:].bitcast(F32)
                        msets = (
                            f32v[:, 0:(head_end + 1) // 2],
                            f32v[:, (tail_beg - 1) // 2:P3 // 2],
                            f32v[:, (mid0 - 1) // 2:(mid0 - 1) // 2 + (H - 1) * (Wp3 // 2)]
                            .rearrange("p (r w) -> p r w", w=Wp3 // 2)[:, :, 0:4],
                        )
                        for reg in msets:
                            nc.vector.memset(reg, 0.0)
                        for reg in (
                            v0t[:, 0:head_end],
                            v0t[:, tail_beg:P3],
                            v0t[:, mid0:mid0 + (H - 1) * Wp3].rearrange(
                                "p (r w) -> p r w", w=Wp3)[:, :, 0:7],
                        ):
                            nc.vector.tensor_scalar_add(reg, reg, ws[padv])
                    xp_hist = {}
                    vg0_hist = {}

                    def emit_c1(ci):
                        c0 = ci * RC
                        nr_c = min(RC, H - c0)
                        NN = nr_c * W
                        o0 = c0 * W
                        # residual stream chunks
                        xpch = [c1r.tile([96, RC * W], BF16, tag=f"cx{cg}",
                                         name=f"cx{cg}", bufs=3) for cg in range(2)]
                        xp_hist[ci] = xpch
                        xcch = [c1r.tile([96, RC * W], BF16, tag=f"cxc{cg}",
                                         name=f"cxc{cg}") for cg in range(2)]
                        for cg in range(2):
                            nc.sync.dma_start(out=xcch[cg][:, :NN],
                                              in_=xc_sp[cg][:, o0:o0 + NN])
                        # attention out chunk
                        av_ps = cps.tile([128, RC * W], F32, tag="avpj", bufs=2)
                        nc.tensor.matmul(av_ps[:, :NN], probsT[:],
                                         v_res[:, o0:o0 + NN], start=True, stop=True)
                        avs = c1r.tile([96, RC * W], BF16, tag="avs")
                        nc.vector.tensor_copy(out=avs[:, :NN], in_=av_ps[:96, :NN])
                        xsq = [c1r.tile([96, RC * W], BF16, tag=f"cs{cg}",
                                        name=f"cs{cg}") for cg in range(2)]
                        for cg in range(2):
                            pj_ps = cps.tile([128, RC * W], F32, tag="avpj", bufs=2)
                            nc.tensor.matmul(pj_ps[:, :NN],
                                             ws["proj2T"][:, cg * 128:(cg + 1) * 128],
                                             yn2res[:, o0:o0 + NN], start=True, stop=False)
                            nc.tensor.matmul(pj_ps[:, :NN],
                                             ws["proj1T"][:, cg * 128:(cg + 1) * 128],
                                             avs[:, :NN], start=False, stop=True)
                            nc.vector.scalar_tensor_tensor(
                                out=xpch[cg][:, :NN], in0=pj_ps[:96, :NN],
                                scalar=ws["proj_bias"][:, cg:cg + 1], in1=xcch[cg][:, :NN],
                                op0=Alu.add, op1=Alu.add)
                            nc.scalar.square(xsq[cg][:, :NN], xpch[cg][:, :NN])
                        # LN2 stats
                        mu_ps = cps.tile([128, RC * W], F32, tag="c1s", bufs=2)
                        m2_ps = cps.tile([128, RC * W], F32, tag="c1s", bufs=2)
                        for cg in range(2):
                            nc.tensor.matmul(mu_ps[:, :NN], ws["ones_st2"], xpch[cg][:, :NN],
                                             start=(cg == 0), stop=(cg == 1))
                            nc.tensor.matmul(m2_ps[:, :NN], ws["ones_st2"], xsq[cg][:, :NN],
                                             start=(cg == 0), stop=(cg == 1))
                        musq = c1r.tile([128, RC * W], F32, tag="cst")
                        nc.scalar.activation(musq[:, :NN], mu_ps[:, :NN], Act.Square,
                                             scale=4.0 / 3.0)
                        var = c1r.tile([128, RC * W], F32, tag="cst")
                        nc.vector.scalar_tensor_tensor(
                            out=var[:, :NN], in0=m2_ps[:, :NN], scalar=4.0 / 3.0,
                            in1=musq[:, :NN], op0=Alu.mult, op1=Alu.subtract)
                        sd = c1r.tile([128, RC * W], F32, tag="cst")
                        nc.scalar.activation(sd[:, :NN], var[:, :NN], Act.Sqrt,
                                             bias=ws["epsv2"])
                        rstd = c1r.tile([128, RC * W], F32, tag="cst")
                        nc.vector.reciprocal_approx_fast(out=rstd[:, :NN], in_=sd[:, :NN])
                        yn2t = [c1r.tile([96, RC * W], BF16, tag=f"cy{cg}",
                                         name=f"cy{cg}") for cg in range(2)]
                        for cg in range(2):
                            td = c1r.tile([96, RC * W], F32, tag="ctd")
                            nc.vector.scalar_tensor_tensor(
                                out=td[:, :NN], in0=mu_ps[:96, :NN], scalar=-4.0 / 3.0,
                                in1=xpch[cg][:, :NN], op0=Alu.mult, op1=Alu.add)
                            nc.vector.tensor_tensor(out=yn2t[cg][:, :NN], in0=td[:, :NN],
                                                    in1=rstd[:96, :NN], op=Alu.mult)
                        base = pd3(3 + c0) + 3

                        def v0dst(v0t, lo, hi, off=0):
                            return v0t[lo:hi, base + off:base + off + nr_c * Wp3] \
                                .rearrange("p (r w) -> p r w", w=Wp3)[:, :, 0:W]

                        vg0 = c1r.tile([64, RC * W], BF16, tag="vg0", bufs=3)
                        vg0_hist[ci] = vg0
                        for mg in range(2):
                            fps = cps.tile([128, RC * W], F32, tag="c1s", bufs=2)
                            for cg in range(2):
                                nc.tensor.matmul(
                                    fps[:, :NN],
                                    ws["fc1T"][:, (mg * 2 + cg) * 128:(mg * 2 + cg + 1) * 128],
                                    yn2t[cg][:, :NN], start=(cg == 0), stop=(cg == 1))
                            fsrc = lambda lo, hi: fps[lo:hi, :NN].rearrange(
                                "p (r w) -> p r w", w=W)
                            if mg == 0:
                                nc.scalar.activation(vg0[:, :NN], fps[0:64, :NN], Act.Gelu,
                                                     bias=ws["fc1_bias"][0:64, 0:1])
                                nc.scalar.activation(v0dst(v0t1, 0, 64), fsrc(64, 128),
                                                     Act.Gelu, bias=ws["fc1_bias"][64:128, 0:1])
                            else:
                                nc.scalar.activation(v0dst(v0t1, 64, 128), fsrc(0, 64),
                                                     Act.Gelu, bias=ws["fc1_bias"][0:64, 1:2])
                                nc.scalar.activation(v0dst(v0t2, 0, 64), fsrc(64, 128),
                                                     Act.Gelu, bias=ws["fc1_bias"][64:128, 1:2])
                                nc.scalar.activation(v0dst(v0t2, 64, 128, off=1), fsrc(64, 128),
                                                     Act.Gelu, bias=ws["fc1_bias"][64:128, 1:2])

                    def emit_c2(ci):
                        c0 = ci * RC
                        nr_c = min(RC, H - c0)
                        N = nr_c * Wp3
                        NN = nr_c * W
                        sb0 = pd3(3 + c0)
                        ps_a = cps.tile([128, RC * Wp3], F32, tag="psa")
                        for t in range(25):
                            dy, dx = t // 5 - 2, t % 5 - 2
                            o = sb0 + dy * Wp3 + dx
                            nc.tensor.matmul(ps_a[:, :N],
                                             ws["pair_diag"][:, t * 128:(t + 1) * 128],
                                             v0t1[:, o:o + N],
                                             start=(t == 0), stop=(t == 24))
                        ps_b = cps.tile([128, RC * Wp3], F32, tag="psb")
                        for i, (dy, dxa, hasb) in enumerate(dw3_passes):
                            o = sb0 + dy * Wp3 + dxa
                            nc.tensor.matmul(ps_b[:, :N],
                                             ws["dw3_diag"][:, i * 128:(i + 1) * 128],
                                             v0t2[:, o:o + N],
                                             start=(i == 0), stop=(i == len(dw3_passes) - 1))

                        def inner(ap_flat, lo, hi):
                            # interior view of a PSUM chunk (starts at free 0)
                            return ap_flat[lo:hi, :N].rearrange(
                                "p (r w) -> p r w", w=Wp3)[:, :, 3:3 + W]

                        def inner_v0(ap_flat, lo, hi):
                            # interior view of the padded v0 buffers at this chunk
                            return ap_flat[lo:hi, sb0:sb0 + N].rearrange(
                                "p (r w) -> p r w", w=Wp3)[:, :, 3:3 + W]

                        ug_a = c2r.tile([128, RC * W], BF16, tag="uga")
                        ug_b = c2r.tile([128, RC * W], BF16, tag="ugb")
                        vb_a = c2r.tile([128, RC * W], BF16, tag="vba")
                        vb_b = c2r.tile([128, RC * W], BF16, tag="vbb")
                        g0v = vg0_hist.pop(ci)
                        nc.scalar.activation(ug_a[0:64, :NN], g0v[:, :NN], Act.Gelu,
                                             bias=ws["g0_bias"], scale=ws["g0_scale"])
                        nc.scalar.activation(
                            ug_a[64:128, :NN].rearrange("p (r w) -> p r w", w=W),
                            inner(ps_a, 0, 64), Act.Gelu, bias=ws["pair_bias"][0:64])
                        nc.scalar.activation(
                            ug_b[0:64, :NN].rearrange("p (r w) -> p r w", w=W),
                            inner(ps_a, 64, 128), Act.Gelu, bias=ws["pair_bias"][64:128])
                        nc.scalar.activation(
                            ug_b[64:128, :NN].rearrange("p (r w) -> p r w", w=W),
                            inner(ps_b, 0, 64), Act.Gelu, bias=ws["dw3_bias"])
                        nc.vector.tensor_scalar(out=vb_a[0:64, :NN], in0=g0v[:, :NN],
                                                scalar1=ws["s1a"][0:64],
                                                scalar2=ws["t1a"][0:64],
                                                op0=Alu.mult, op1=Alu.add)
                        nc.vector.tensor_scalar(out=vb_a[64:128, :NN],
                                                in0=inner_v0(v0t1, 0, 64),
                                                scalar1=ws["s1a"][64:128],
                                                scalar2=ws["t1a"][64:128],
                                                op0=Alu.mult, op1=Alu.add)
                        nc.vector.tensor_scalar(out=vb_b[0:64, :NN],
                                                in0=inner_v0(v0t1, 64, 128),
                                                scalar1=ws["s1b"][0:64],
                                                scalar2=ws["t1b"][0:64],
                                                op0=Alu.mult, op1=Alu.add)
                        nc.vector.tensor_scalar(out=vb_b[64:128, :NN],
                                                in0=inner_v0(v0t2, 0, 64),
                                                scalar1=ws["s1b"][64:128],
                                                scalar2=ws["t1b"][64:128],
                                                op0=Alu.mult, op1=Alu.add)
                        z1a = c2r.tile([128, RC * W], BF16, tag="z1a")
                        z1b = c2r.tile([128, RC * W], BF16, tag="z1b")
                        nc.vector.tensor_tensor(out=z1a[:, :NN], in0=ug_a[:, :NN],
                                                in1=vb_a[:, :NN], op=Alu.mult)
                        nc.vector.tensor_tensor(out=z1b[:, :NN], in0=ug_b[:, :NN],
                                                in1=vb_b[:, :NN], op=Alu.mult)
                        xpres = xp_hist.pop(ci)
                        for cg in range(2):
                            ops = cps.tile([128, RC * W], F32, tag="ops", bufs=2)
                            nc.tensor.matmul(ops[:, :NN],
                                             ws["fc2aT"][:, (cg * 2) * 128:(cg * 2 + 1) * 128],
                                             z1a[:, :NN], start=True, stop=False)
                            nc.tensor.matmul(ops[:, :NN],
                                             ws["fc2aT"][:, (cg * 2 + 1) * 128:(cg * 2 + 2) * 128],
                                             z1b[:, :NN], start=False, stop=False)
                            nc.tensor.matmul(ops[:, :NN],
                                             ws["fc2bvA"][:, cg * 128:(cg + 1) * 128],
                                             vb_a[:, :NN], start=False, stop=False)
                            nc.tensor.matmul(ops[:, :NN],
                                             ws["fc2bvB"][:, cg * 128:(cg + 1) * 128],
                                             vb_b[:, :NN], start=False, stop=True)
                            ob = c2r.tile([96, RC * W], F32, tag=f"ob{cg}", bufs=1)
                            nc.vector.tensor_scalar(out=ob[:, :NN], in0=ops[:96, :NN],
                                                    scalar1=ws["s3v"][:, cg:cg + 1],
                                                    scalar2=ws["out_bias"][:, cg:cg + 1],
                                                    op0=Alu.mult, op1=Alu.add)
                            oc = c2r.tile([96, RC * W], F32, tag=f"oc{cg}", bufs=1)
                            nc.vector.tensor_tensor(out=oc[:, :NN], in0=ob[:, :NN],
                                                    in1=xpres[cg][:, :NN], op=Alu.add)
                            nc.sync.dma_start(
                                out=out_t[cg * 96:(cg + 1) * 96, c0 * W:c0 * W + NN],
                                in_=oc[:, :NN])

                    for ci in range(NCH + 2):
                        if ci < NCH:
                            emit_c1(ci)
                        if ci >= 2:
                            emit_c2(ci - 2)
            _wpc_cm.__exit__(None, None, None)
    return out_t.name


# ----------------------------------------------------------------------------
# host entry
# ----------------------------------------------------------------------------

_CACHE = {}


def make_program(H, W, n_cores, attn_scale, dw3_passes):
    key = (H, W, n_cores, round(attn_scale, 9))
    if key in _CACHE:
        return _CACHE[key]
    nc = bacc.Bacc("TRN2", target_bir_lowering=False, debug=False, num_devices=n_cores)
    out_name = build(nc, H, W, n_cores, attn_scale, dw3_passes)
    nc.compile()
    _CACHE[key] = (nc, out_name)
    return nc, out_name


def make_in_maps(inputs):
    import ml_dtypes
    x = np.asarray(inputs["x"], np.float32)
    B = x.shape[0]
    wdict = _prep_weights({k: np.asarray(v) for k, v in inputs.items()})
    base = {}
    for k, (shp, d) in WSPEC.items():
        base["w_" + k] = wdict[k][0].reshape(shp)
    in_maps = []
    for b in range(B):
        m = dict(base)
        m["x"] = np.ascontiguousarray(x[b].reshape(-1, x.shape[-1]).T) \
            .astype(np.float16)
        in_maps.append(m)
    return in_maps, wdict


def kernel(**inputs):
    x = np.asarray(inputs["x"], np.float32)
    B, H, W, C = x.shape
    in_maps, wdict = make_in_maps(inputs)
    nc, out_name = make_program(H, W, B, wdict["_attn_scale"][0],
                                wdict["_dw3_passes"][0])
    res = bass_utils.run_bass_kernel_spmd(nc, in_maps, core_ids=list(range(B)))
    return np.stack([res.results[b][out_name].reshape(C, H, W).transpose(1, 2, 0)
                     for b in range(B)])
